# revision 11
# baseline (speedup 1.0000x reference)
"""Multi-head attention (B=2, S=2048, D=1024, H=16) on 8 Trainium2 cores.

Sharding: core = 4*b + g  (b = batch 0..1, g = head-group 0..3, 4 heads each).
Heads are processed in pairs; pair p covers the group's d-dims [128p, 128p+128).

Schedule: 64 pipelined rounds (one per (pair, qc, k-group)) keep the scalar
engine's exp stream and the tensor engine concurrently busy:

  round r: [normalize-b] [forced proj] scores(r) -> exp(r) -> PV(r-1)
           [normalize-a at sweep end] [V jit] [filler: proj / out-proj]

A short DMA-led preamble projects only K(pair0 sc0/sc1), Q(pair0,qc0),
V(pair0 kt0,1); all other projections and the output projection run as PE
filler in the rounds' slack so the tensor engine stays busy at full DVFS
pstate.  The sweep order interleaves pairs/q-chunks so output tiles become
ready (and their HBM writes drain) from mid-kernel onward instead of
piling into a tail.  Normalize is split into two phases one round apart so
its z-gather DMA never head-blocks the vector-engine queue.  The last
q-chunk's output projection is split by head-pair so only its pair1 half
remains after the final exp.

Exactness notes: b_k only shifts each softmax row uniformly -> dropped.
b_v and b_o commute with softmax-average -> folded into the host reduce.
b_q is applied on-device (fused into the Q PSUM->SBUF copy).
"""

import os
from collections import defaultdict, deque
from contextlib import ExitStack

import ml_dtypes
import numpy as np

import concourse.tile as tile
from concourse import bacc, mybir

B, S, D = 2, 2048, 1024
H, DH = 16, 64
NCORES = 8
NG = 4                  # head-group shards
DG = D // NG            # 256 dims per head-group (4 heads, 2 pairs)
P = 128
QC = 512                # q-chunk width
NQC = S // QC           # 4
NKT = S // P            # 16 k-tiles of 128
CD = D // P             # 8 contraction tiles for the projections
NR = 64                 # pipeline rounds
# sweep order: (pair, qc) interleaved so outputs complete throughout
SW = [(0, 0), (0, 1), (1, 0), (1, 1), (0, 2), (1, 2), (0, 3), (1, 3)]
F32 = mybir.dt.float32
BF16 = mybir.dt.bfloat16
AF = mybir.ActivationFunctionType
SCALE = 1.0 / float(np.sqrt(D))


def _body(ctx: ExitStack, tc: "tile.TileContext", io: dict):
    nc = tc.nc
    ctx.enter_context(nc.allow_low_precision(reason="bf16 matmul pipeline"))
    sb = ctx.enter_context(tc.tile_pool(name="sb", bufs=1))
    ps = ctx.enter_context(tc.tile_pool(name="ps", bufs=1, space="PSUM"))

    # ---------------- DMA: one ordered chain, earliest-needed first --------
    xk_sb, xq_sb, xv_sb = {}, {}, {}
    w_sb = {}

    def dma_x(dst_map, key, idx, halves=False):
        t = sb.tile([P, CD, QC], BF16, tag="x", bufs=12, name=f"{key}{idx}")
        if halves:
            nc.sync.dma_start(t[:, 0:4, :], io[key][idx, :, 0:4, :])
            nc.sync.dma_start(t[:, 4:8, :], io[key][idx, :, 4:8, :])
        else:
            nc.sync.dma_start(t[:], io[key][idx])
        dst_map[idx] = t

    def dma_w(kind, pr):
        t = sb.tile([P, CD, P], BF16, tag="w", bufs=5, name=f"w{kind}{pr}")
        nc.sync.dma_start(t[:], io[f"w{kind}"][pr])
        w_sb[(kind, pr)] = t

    dma_w("k", 0)
    dma_x(xk_sb, "xk", 0, halves=True)
    dma_w("q", 0)
    bq = sb.tile([P, 2], F32, tag="bq", bufs=1, name="bq")
    nc.sync.dma_start(bq[:], io["bq"])
    ones2 = sb.tile([P, 2], BF16, tag="ones2", bufs=1, name="ones2")
    nc.sync.dma_start(ones2[:], io["ones2"])
    dma_x(xq_sb, "xq", 0, halves=True)
    dma_x(xk_sb, "xk", 1)
    dma_w("k", 1)
    wv = sb.tile([P, CD, DG], BF16, tag="wv", bufs=1, name="wv")
    nc.sync.dma_start(wv[:], io["wv"])
    dma_x(xv_sb, "xv", 0)
    dma_x(xv_sb, "xv", 1)
    dma_x(xk_sb, "xk", 2)
    dma_x(xv_sb, "xv", 2)
    dma_x(xk_sb, "xk", 3)
    dma_x(xv_sb, "xv", 3)
    dma_x(xq_sb, "xq", 1)
    dma_w("q", 1)
    dma_x(xq_sb, "xq", 2)
    dma_x(xq_sb, "xq", 3)
    woT = []
    for pr in range(2):
        t = sb.tile([P, D], BF16, tag="wo", bufs=2, name=f"woT{pr}")
        nc.sync.dma_start(t[:], io["wo"][pr])
        woT.append(t)

    # ---------------- projection / out-proj emitters -----------------------
    KT, QT, V, UN, YSB = {}, {}, {}, {}, {}
    pgroups = {}

    def qk_part_a(kind, pr, idx):
        w = w_sb[(kind, pr)]
        x = (xk_sb if kind == "k" else xq_sb)[idx]
        pg = ps.tile([P, QC], F32, tag="pj", bufs=1, name=f"pg{kind}{pr}{idx}")
        for c in range(4):
            nc.tensor.matmul(
                pg[:], w[:, c, :], x[:, c, :], start=(c == 0), stop=False
            )
        pgroups[(kind, pr, idx)] = pg

    def qk_part_b(kind, pr, idx):
        w = w_sb[(kind, pr)]
        x = (xk_sb if kind == "k" else xq_sb)[idx]
        pg = pgroups.pop((kind, pr, idx))
        for c in range(4, CD):
            nc.tensor.matmul(
                pg[:], w[:, c, :], x[:, c, :], start=False, stop=(c == CD - 1)
            )
        t = sb.tile([P, QC], BF16, tag=f"{kind}t", bufs=8, name=f"{kind}T{pr}_{idx}")
        if kind == "q":
            nc.vector.tensor_scalar_add(t[:], pg[:], bq[:, pr : pr + 1])
            QT[(pr, idx)] = t
        else:
            nc.vector.tensor_copy(t[:], pg[:])
            KT[(pr, idx)] = t

    def emit_v_group(pair, kt):
        sc, off = divmod(kt, 4)
        x = xv_sb[sc]
        pg = ps.tile([P, P], F32, tag="pj", bufs=1, name=f"pgv{pair}{kt}")
        for c in range(CD):
            nc.tensor.matmul(
                pg[:],
                x[:, c, off * P : (off + 1) * P],
                wv[:, c, pair * P : (pair + 1) * P],
                start=(c == 0),
                stop=(c == CD - 1),
            )
        vt = sb.tile([P, 2, DH + 1], BF16, tag="v", bufs=32, name=f"V{pair}_{kt}")
        nc.vector.tensor_copy(
            vt[:, :, 0:DH], pg[:].rearrange("p (i d) -> p i d", i=2)
        )
        nc.vector.tensor_copy(vt[:, :, DH : DH + 1], ones2[:, :, None])
        V[(pair, kt)] = vt

    pending = deque()

    def emit_outproj_unit(tag="pj"):
        qc, qi, ec = pending.popleft()
        qt = qc * 4 + qi
        if ec == 0:
            YSB[qt] = sb.tile([P, D], BF16, tag="y", bufs=4, name=f"Y{qt}")
        ysb = YSB[qt]
        yp = ps.tile(
            [P, QC], F32, tag=tag, bufs=(1 if tag == "pj" else 3), name=f"yp{qt}_{ec}"
        )
        for pr in range(2):
            nc.tensor.matmul(
                yp[:],
                UN[(qc, pr)][:, qi * P : (qi + 1) * P],
                woT[pr][:, ec * QC : (ec + 1) * QC],
                start=(pr == 0),
                stop=(pr == 1),
            )
        nc.vector.tensor_copy(ysb[:, ec * QC : (ec + 1) * QC], yp[:])
        nc.sync.dma_start(
            io["y"][qt * P : (qt + 1) * P, ec * QC : (ec + 1) * QC],
            ysb[:, ec * QC : (ec + 1) * QC],
        )

    half_pending = deque()

    def emit_outproj_half(tag="pj"):
        # pair0 half of a qc3 unit: bf16 partial kept in SBUF
        qi, ec = half_pending.popleft()
        qt = 12 + qi
        if ec == 0:
            YSB[qt] = sb.tile([P, D], BF16, tag="y3", bufs=4, name=f"YP{qt}")
        yp = ps.tile(
            [P, QC], F32, tag=tag, bufs=(1 if tag == "pj" else 3), name=f"yh{qt}_{ec}"
        )
        nc.tensor.matmul(
            yp[:],
            UN[(3, 0)][:, qi * P : (qi + 1) * P],
            woT[0][:, ec * QC : (ec + 1) * QC],
            start=True,
            stop=True,
        )
        nc.vector.tensor_copy(YSB[qt][:, ec * QC : (ec + 1) * QC], yp[:])

    def emit_outproj_complete(qi, ec, tag):
        qt = 12 + qi
        ysb = YSB[qt]
        yp = ps.tile(
            [P, QC], F32, tag=tag, bufs=(1 if tag == "pj" else 3), name=f"yc{qt}_{ec}"
        )
        nc.tensor.matmul(
            yp[:],
            UN[(3, 1)][:, qi * P : (qi + 1) * P],
            woT[1][:, ec * QC : (ec + 1) * QC],
            start=True,
            stop=True,
        )
        nc.vector.tensor_add(
            ysb[:, ec * QC : (ec + 1) * QC],
            yp[:],
            ysb[:, ec * QC : (ec + 1) * QC],
        )
        nc.sync.dma_start(
            io["y"][qt * P : (qt + 1) * P, ec * QC : (ec + 1) * QC],
            ysb[:, ec * QC : (ec + 1) * QC],
        )

    # ---------------- attention round emitters -----------------------------
    PTs, U = {}, {}
    ZT = {}

    def sweep_of(r):
        s = r // 8
        pair, qc = SW[s]
        return s, pair, qc, r % 8

    def emit_scores(r):
        _, pair, qc, kg = sweep_of(r)
        for i in (0, 1):
            lo = 64 * i
            st = ps.tile([P, 2, QC], F32, tag="st", bufs=2, name=f"st{r}_{i}")
            for kk in (0, 1):
                kt = kg * 2 + kk
                sc, off = divmod(kt, 4)
                nc.tensor.matmul(
                    st[:, kk, :],
                    KT[(pair, sc)][lo : lo + 64, off * P : (off + 1) * P],
                    QT[(pair, qc)][lo : lo + 64, :],
                    start=True,
                    stop=True,
                    tile_position=(lo, 0),
                )
            pt = sb.tile([P, 2, QC], BF16, tag="pt", bufs=6, name=f"pt{r}_{i}")
            nc.scalar.activation(pt[:], st[:], AF.Exp, scale=SCALE)
            PTs[(r, i)] = pt

    def emit_pv(r):
        s, pair, qc, kg = sweep_of(r)
        if kg == 0:
            U[s] = [
                ps.tile([P, QC], F32, tag="u", bufs=3, name=f"U{s}_{i}")
                for i in (0, 1)
            ]
        for i in (0, 1):
            pt = PTs.pop((r, i))
            for kk in (0, 1):
                kt = kg * 2 + kk
                nc.tensor.matmul(
                    U[s][i][0:65, :],
                    V[(pair, kt)][:, i, :],
                    pt[:, kk, :],
                    start=(kg == 0 and kk == 0),
                    stop=(kg == 7 and kk == 1),
                )

    def emit_normalize_a(s):
        for i in (0, 1):
            zr = sb.tile([65, QC], F32, tag="zr", bufs=2, name=f"zr{s}_{i}")
            nc.vector.tensor_copy(zr[64:65, :], U[s][i][64:65, :])
            z = sb.tile([1, QC], F32, tag="z", bufs=2, name=f"z{s}_{i}")
            nc.sync.dma_start(z[:], zr[64:65, :])
            ZT[(s, i)] = z

    def emit_normalize_b(s):
        pair, qc = SW[s]
        un = sb.tile([P, QC], BF16, tag="un", bufs=8, name=f"UN{qc}_{pair}")
        for i in (0, 1):
            z = ZT.pop((s, i))
            rz = sb.tile([1, QC], F32, tag="rz", bufs=2, name=f"rz{s}_{i}")
            nc.vector.reciprocal(rz[:], z[:])
            rb = sb.tile([64, QC], F32, tag="rb", bufs=2, name=f"rb{s}_{i}")
            nc.gpsimd.partition_broadcast(rb[:], rz[:], channels=64)
            if i == 0:
                nc.vector.tensor_mul(un[0:64, :], U[s][i][0:64, :], rb[:])
            else:
                tmp = sb.tile([64, QC], BF16, tag="untmp", bufs=2, name=f"ut{s}")
                nc.vector.tensor_mul(tmp[:], U[s][i][0:64, :], rb[:])
                nc.sync.dma_start(un[64:128, :], tmp[:])
        del U[s]
        UN[(qc, pair)] = un
        if (qc, 1 - pair) in UN:
            if qc < 3:
                pending.extend((qc, qi, ec) for qi in range(4) for ec in range(2))
        if pair == 0 and qc == 3:
            half_pending.extend((qi, ec) for qi in range(4) for ec in range(2))

    # ---------------- static schedule --------------------------------------
    class Job:
        __slots__ = ("cols", "fn", "done")

        def __init__(self, cols, fn):
            self.cols, self.fn, self.done = cols, fn, False

        def run(self):
            if not self.done:
                self.done = True
                self.fn()

    def qk_group(kind, pr, idx):
        qk_part_a(kind, pr, idx)
        qk_part_b(kind, pr, idx)

    jobs = {}
    for kind, pr, idx in [
        ("q", 0, 1), ("q", 0, 2), ("q", 0, 3),
        ("k", 0, 2), ("k", 0, 3),
        ("k", 1, 0), ("k", 1, 1), ("k", 1, 2), ("k", 1, 3),
        ("q", 1, 0), ("q", 1, 1), ("q", 1, 2), ("q", 1, 3),
    ]:
        jobs[(kind, pr, idx)] = Job(
            4096, (lambda a: (lambda: qk_group(*a)))((kind, pr, idx))
        )

    half_specs = [(qi, ec) for qi in range(4) for ec in range(2)]
    half_jobs = [Job(512, emit_outproj_half) for _ in range(8)]

    # EDF-ordered filler queue with earliest-emission gates (DMA arrival).
    fq = deque(
        [
            (7, jobs[("q", 0, 1)]),
            (8, jobs[("k", 1, 0)]),
            (9, jobs[("q", 1, 0)]),
            (9, jobs[("k", 1, 1)]),
            (10, jobs[("k", 1, 2)]),
            (11, jobs[("k", 1, 3)]),
            (12, jobs[("q", 1, 1)]),
            (13, jobs[("q", 0, 2)]),
            (14, jobs[("q", 1, 2)]),
            (15, jobs[("q", 0, 3)]),
            (16, jobs[("q", 1, 3)]),
        ]
        + [(57, hj) for hj in half_jobs]
    )

    # mand_pre: tiles this round's scores read -> force before scores.
    mand_pre = defaultdict(list)
    mand_pre[2].append(jobs[("k", 0, 2)])
    mand_pre[4].append(jobs[("k", 0, 3)])
    mand_pre[8].append(jobs[("q", 0, 1)])
    mand_pre[16].append(jobs[("k", 1, 0)])
    mand_pre[16].append(jobs[("q", 1, 0)])
    mand_pre[18].append(jobs[("k", 1, 1)])
    mand_pre[20].append(jobs[("k", 1, 2)])
    mand_pre[22].append(jobs[("k", 1, 3)])
    mand_pre[24].append(jobs[("q", 1, 1)])
    mand_pre[32].append(jobs[("q", 0, 2)])
    mand_pre[40].append(jobs[("q", 1, 2)])
    mand_pre[48].append(jobs[("q", 0, 3)])
    mand_pre[56].append(jobs[("q", 1, 3)])

    # mand_post: V just-in-time; pair0 rounds 0-6, pair1 rounds 7-14.
    mand_post = defaultdict(list)
    for r in range(7):
        mand_post[r].extend(
            Job(1024, (lambda k: (lambda: emit_v_group(0, k)))(kt))
            for kt in (2 * r + 2, 2 * r + 3)
        )
    for j, r in enumerate(range(7, 15)):
        mand_post[r].extend(
            Job(1024, (lambda k: (lambda: emit_v_group(1, k)))(kt))
            for kt in (2 * j, 2 * j + 1)
        )

    # ---------------- preamble ---------------------------------------------
    qk_part_a("k", 0, 0)
    qk_part_b("k", 0, 0)
    qk_part_a("k", 0, 1)
    qk_part_b("k", 0, 1)
    qk_part_a("q", 0, 0)
    qk_part_b("q", 0, 0)
    emit_v_group(0, 0)
    emit_v_group(0, 1)

    # ---------------- main pipeline ----------------------------------------
    BUDGET = 2600
    for r in range(NR):
        if r % 8 == 1 and r > 8:
            emit_normalize_b(r // 8 - 1)
        for job in mand_pre[r]:
            job.run()
        emit_scores(r)
        if r > 0:
            emit_pv(r - 1)
        if r % 8 == 0 and r > 0:
            emit_normalize_a(r // 8 - 1)
        for job in mand_post[r]:
            job.run()
        budget = BUDGET
        while budget > 0:
            while fq and fq[0][1].done:
                fq.popleft()
            if fq and fq[0][0] <= r:
                _, job = fq.popleft()
                budget -= job.cols
                job.run()
            elif pending:
                emit_outproj_unit()
                budget -= 1024
            else:
                break

    # ---------------- drain -------------------------------------------------
    emit_pv(NR - 1)
    emit_normalize_a(7)
    for _, job in fq:
        job.run()
    while pending:
        emit_outproj_unit()
    while half_pending:
        emit_outproj_half()
    emit_normalize_b(7)
    tags = ["pj", "u", "u", "u"]
    for n, (qi, ec) in enumerate(half_specs):
        emit_outproj_complete(qi, ec, tags[n % 4])


def build_program():
    nc = bacc.Bacc(
        "TRN2", target_bir_lowering=False, debug=False, num_devices=NCORES
    )
    io = {
        "xq": nc.dram_tensor("xq", [NQC, P, CD, QC], BF16, kind="ExternalInput").ap(),
        "xk": nc.dram_tensor("xk", [NQC, P, CD, QC], BF16, kind="ExternalInput").ap(),
        "xv": nc.dram_tensor("xv", [NQC, P, CD, QC], BF16, kind="ExternalInput").ap(),
        "wq": nc.dram_tensor("wq", [2, P, CD, P], BF16, kind="ExternalInput").ap(),
        "wk": nc.dram_tensor("wk", [2, P, CD, P], BF16, kind="ExternalInput").ap(),
        "wv": nc.dram_tensor("wv", [P, CD, DG], BF16, kind="ExternalInput").ap(),
        "wo": nc.dram_tensor("wo", [2, P, D], BF16, kind="ExternalInput").ap(),
        "bq": nc.dram_tensor("bq", [P, 2], F32, kind="ExternalInput").ap(),
        "ones2": nc.dram_tensor("ones2", [P, 2], BF16, kind="ExternalInput").ap(),
        "y": nc.dram_tensor("y", [S, D], BF16, kind="ExternalOutput").ap(),
    }
    with tile.TileContext(nc) as tc:
        with ExitStack() as ctx:
            _body(ctx, tc, io)
    nc.compile()
    return nc


_CACHE = {}


def _get_program():
    if "nc" not in _CACHE:
        _CACHE["nc"] = build_program()
    return _CACHE["nc"]


def make_in_maps(inputs):
    q = np.asarray(inputs["query"], np.float32)
    k = np.asarray(inputs["key"], np.float32)
    v = np.asarray(inputs["value"], np.float32)
    W_q = np.asarray(inputs["W_q"], np.float32)
    W_k = np.asarray(inputs["W_k"], np.float32)
    W_v = np.asarray(inputs["W_v"], np.float32)
    W_o = np.asarray(inputs["W_o"], np.float32)
    b_q = np.asarray(inputs["b_q"], np.float32)

    bf = ml_dtypes.bfloat16

    def xblocks(x):  # [S, D] activations -> [blk, p, c, s] with x.T blocked
        xt = np.ascontiguousarray(x.T).astype(bf)  # [D, S]
        return np.ascontiguousarray(
            xt.reshape(CD, P, NQC, QC).transpose(2, 1, 0, 3)
        )

    def wblocks(w_sl):  # [D, 256] (= W[sl].T) -> [pr, p, c, d]
        return np.ascontiguousarray(
            w_sl.reshape(CD, P, 2, P).transpose(2, 1, 0, 3).astype(bf)
        )

    xq = [xblocks(q[b]) for b in range(B)]
    xk = [xblocks(k[b]) for b in range(B)]
    xv = [xblocks(v[b]) for b in range(B)]

    in_maps = []
    for core in range(NCORES):
        b, g = divmod(core, NG)
        sl = slice(g * DG, (g + 1) * DG)
        in_maps.append(
            {
                "xq": xq[b],
                "xk": xk[b],
                "xv": xv[b],
                "wq": wblocks(W_q[sl, :].T),
                "wk": wblocks(W_k[sl, :].T),
                "wv": np.ascontiguousarray(
                    W_v[sl, :].T.reshape(CD, P, DG).transpose(1, 0, 2).astype(bf)
                ),
                "wo": np.ascontiguousarray(
                    W_o[:, sl].T.reshape(2, P, D).astype(bf)
                ),
                "bq": np.ascontiguousarray(b_q[sl].reshape(2, P).T),
                "ones2": np.ones((P, 2), bf),
            }
        )
    return in_maps


def kernel(**inputs):
    from concourse.bass_utils import run_bass_kernel_spmd

    nc = _get_program()
    in_maps = make_in_maps(inputs)
    trace = bool(int(os.environ.get("MHA_TRACE", "0")))
    res = run_bass_kernel_spmd(nc, in_maps, list(range(NCORES)), trace=trace)
    _CACHE["last_results"] = res

    W_o = np.asarray(inputs["W_o"], np.float64)
    b_o = np.asarray(inputs["b_o"], np.float64)
    b_v = np.asarray(inputs["b_v"], np.float64)
    out = np.zeros((B, S, D), np.float32)
    for core in range(NCORES):
        b = core // NG
        out[b] += res.results[core]["y"].astype(np.float32)
    # b_v and b_o commute with the attention average / output projection.
    out += (b_o + b_v @ W_o.T).astype(np.float32)[None, None, :]
    return out


# revision 13
# speedup vs baseline: 1.1015x; 1.1015x over previous
"""Multi-head attention (B=2, S=2048, D=1024, H=16) on 8 Trainium2 cores.

Sharding: core = 4*b + g  (b = batch 0..1, g = head-group 0..3, 4 heads each).
Heads are processed in pairs; pair p covers the group's d-dims [128p, 128p+128).

Schedule: the scalar engine's exp stream (128 activations of [128,1024],
~172us) is the critical resource.  A short DMA-led preamble projects only
K(pair0), Q(pair0,qc0), V(kt0,1); then 64 pipelined rounds (one per
(pair, qc, k-group)) keep ACT continuously busy:

  round r: [forced proj groups] scores(r) -> exp(r) -> PV(r-1)
           [normalize at sweep boundaries] [filler: proj / out-proj]

All other projections (K pair1, remaining Q, V) and the output projection
run as PE filler inside the rounds' slack so the tensor engine never idles
(and stays at full DVFS pstate).  Sweep order is pair-major so pair1's
weights/projections have 4 sweeps of slack to materialize.

Exactness notes: b_k only shifts each softmax row uniformly -> dropped.
b_v and b_o commute with softmax-average -> folded into the host reduce.
b_q is applied on-device (fused into the Q PSUM->SBUF copy).
"""

import os
from collections import defaultdict, deque
from contextlib import ExitStack

import ml_dtypes
import numpy as np

import concourse.bass as bass
import concourse.tile as tile
from concourse import bacc, mybir

B, S, D = 2, 2048, 1024
H, DH = 16, 64
NCORES = 8
NG = 4                  # head-group shards
DG = D // NG            # 256 dims per head-group (4 heads, 2 pairs)
P = 128
QC = 512                # q-chunk width
NQC = S // QC           # 4
NKT = S // P            # 16 k-tiles of 128
CD = D // P             # 8 contraction tiles for the projections
NR = 64                 # pipeline rounds: 2 pairs x 4 qc x 8 k-groups
F32 = mybir.dt.float32
BF16 = mybir.dt.bfloat16
AF = mybir.ActivationFunctionType
SCALE = 1.0 / float(np.sqrt(D))


def _body(ctx: ExitStack, tc: "tile.TileContext", io: dict):
    nc = tc.nc
    ctx.enter_context(nc.allow_low_precision(reason="bf16 matmul pipeline"))
    sb = ctx.enter_context(tc.tile_pool(name="sb", bufs=1))
    ps = ctx.enter_context(tc.tile_pool(name="ps", bufs=1, space="PSUM"))

    # ---------------- DMA: inputs stream in consumption order --------------
    xk_sb, xq_sb, xv_sb = {}, {}, {}
    w_sb = {}

    def dma_x(dst_map, key, idx):
        t = sb.tile([P, CD, QC], BF16, tag="x", bufs=12, name=f"{key}{idx}")
        # two halves so projection groups can start on the first half
        nc.sync.dma_start(t[:, 0:4, :], io[key][idx, :, 0:4, :])
        nc.sync.dma_start(t[:, 4:8, :], io[key][idx, :, 4:8, :])
        dst_map[idx] = t

    def dma_w(kind, pr):
        t = sb.tile([P, CD, P], BF16, tag="w", bufs=6, name=f"w{kind}{pr}")
        nc.sync.dma_start(t[:], io[f"w{kind}"][pr])
        w_sb[(kind, pr)] = t

    dma_w("k", 0)
    dma_x(xk_sb, "xk", 0)
    dma_w("q", 0)
    bq = sb.tile([P, 2], F32, tag="bq", bufs=1, name="bq")
    nc.sync.dma_start(bq[:], io["bq"])
    ones2 = sb.tile([P, 2], BF16, tag="ones2", bufs=1, name="ones2")
    nc.sync.dma_start(ones2[:], io["ones2"])
    dma_x(xk_sb, "xk", 1)
    dma_x(xq_sb, "xq", 0)
    dma_w("v", 0)
    dma_x(xv_sb, "xv", 0)
    dma_x(xv_sb, "xv", 1)
    dma_x(xq_sb, "xq", 1)
    dma_x(xk_sb, "xk", 2)
    dma_x(xv_sb, "xv", 2)
    dma_x(xk_sb, "xk", 3)
    dma_x(xv_sb, "xv", 3)
    dma_w("k", 1)
    dma_w("q", 1)
    dma_x(xq_sb, "xq", 2)
    dma_w("v", 1)
    dma_x(xq_sb, "xq", 3)
    woT = []
    for pr in range(2):
        t = sb.tile([P, D], BF16, tag="wo", bufs=2, name=f"woT{pr}")
        nc.sync.dma_start(t[:], io["wo"][pr])
        woT.append(t)

    # ---------------- projection / out-proj emitters -----------------------
    KT, QT, V, UN, YSB = {}, {}, {}, {}, {}

    def emit_qk_group(kind, pr, idx):
        w = w_sb[(kind, pr)]
        x = (xk_sb if kind == "k" else xq_sb)[idx]
        pg = ps.tile([P, QC], F32, tag="pj", bufs=1, name=f"pg{kind}{pr}{idx}")
        for c in range(CD):
            nc.tensor.matmul(
                pg[:], w[:, c, :], x[:, c, :], start=(c == 0), stop=(c == CD - 1)
            )
        t = sb.tile([P, QC], BF16, tag=f"{kind}t", bufs=8, name=f"{kind}T{pr}_{idx}")
        if kind == "q":
            nc.vector.tensor_scalar_add(t[:], pg[:], bq[:, pr : pr + 1])
            QT[(pr, idx)] = t
        else:
            nc.vector.tensor_copy(t[:], pg[:])
            KT[(pr, idx)] = t

    def emit_v_group(pair, kt):
        sc, off = divmod(kt, 4)
        x = xv_sb[sc]
        pg = ps.tile([P, P], F32, tag="pj", bufs=1, name=f"pgv{pair}{kt}")
        for c in range(CD):
            nc.tensor.matmul(
                pg[:],
                x[:, c, off * P : (off + 1) * P],
                w_sb[("v", pair)][:, c, :],
                start=(c == 0),
                stop=(c == CD - 1),
            )
        vt = sb.tile([P, 2, DH + 1], BF16, tag="v", bufs=32, name=f"V{pair}_{kt}")
        nc.vector.tensor_copy(vt[:, :, 0:DH], pg[:].rearrange("p (i d) -> p i d", i=2))
        nc.vector.tensor_copy(vt[:, :, DH : DH + 1], ones2[:, :, None])
        V[(pair, kt)] = vt

    pending = deque()

    def emit_outproj_unit(tag="pj"):
        qc, qi, ec = pending.popleft()
        qt = qc * 4 + qi
        if ec == 0:
            YSB[qt] = sb.tile([P, D], BF16, tag="y", bufs=4, name=f"Y{qt}")
        ysb = YSB[qt]
        yp = ps.tile([P, QC], F32, tag=tag, bufs=(1 if tag == "pj" else 3), name=f"yp{qt}_{ec}")
        for pr in range(2):
            nc.tensor.matmul(
                yp[:],
                UN[(qc, pr)][:, qi * P : (qi + 1) * P],
                woT[pr][:, ec * QC : (ec + 1) * QC],
                start=(pr == 0),
                stop=(pr == 1),
            )
        nc.vector.tensor_copy(ysb[:, ec * QC : (ec + 1) * QC], yp[:])
        if ec == 1:
            nc.sync.dma_start(io["y"][qt * P : (qt + 1) * P, :], ysb[:])

    # ---------------- attention round emitters -----------------------------
    PTs, U = {}, {}

    def sweep_of(r):
        s = r // 8
        pair, qc = divmod(s, 4)
        return s, pair, qc, r % 8

    def emit_scores(r):
        _, pair, qc, kg = sweep_of(r)
        for i in (0, 1):
            lo = 64 * i
            st = ps.tile([P, 2, QC], F32, tag="st", bufs=2, name=f"st{r}_{i}")
            for kk in (0, 1):
                kt = kg * 2 + kk
                sc, off = divmod(kt, 4)
                nc.tensor.matmul(
                    st[:, kk, :],
                    KT[(pair, sc)][lo : lo + 64, off * P : (off + 1) * P],
                    QT[(pair, qc)][lo : lo + 64, :],
                    start=True,
                    stop=True,
                    tile_position=(lo, 0),
                )
            pt = sb.tile([P, 2, QC], BF16, tag="pt", bufs=4, name=f"pt{r}_{i}")
            nc.scalar.activation(pt[:], st[:], AF.Exp, scale=SCALE)
            PTs[(r, i)] = pt

    def emit_pv(r):
        s, pair, qc, kg = sweep_of(r)
        if kg == 0:
            U[s] = [
                ps.tile([P, QC], F32, tag="u", bufs=3, name=f"U{s}_{i}") for i in (0, 1)
            ]
        for i in (0, 1):
            pt = PTs.pop((r, i))
            for kk in (0, 1):
                kt = kg * 2 + kk
                nc.tensor.matmul(
                    U[s][i][0:65, :],
                    V[(pair, kt)][:, i, :],
                    pt[:, kk, :],
                    start=(kg == 0 and kk == 0),
                    stop=(kg == 7 and kk == 1),
                )

    def emit_normalize(s):
        pair, qc = divmod(s, 4)
        un = sb.tile([P, QC], BF16, tag="un", bufs=8, name=f"UN{qc}_{pair}")
        for i in (0, 1):
            u = U[s][i]
            zr = sb.tile([65, QC], F32, tag="zr", bufs=4, name=f"zr{s}_{i}")
            nc.vector.tensor_copy(zr[64:65, :], u[64:65, :])
            z = sb.tile([1, QC], F32, tag="z", bufs=4, name=f"z{s}_{i}")
            nc.sync.dma_start(z[:], zr[64:65, :])
            rz = sb.tile([1, QC], F32, tag="rz", bufs=4, name=f"rz{s}_{i}")
            nc.vector.reciprocal(rz[:], z[:])
            rb = sb.tile([64, QC], F32, tag="rb", bufs=4, name=f"rb{s}_{i}")
            nc.gpsimd.partition_broadcast(rb[:], rz[:], channels=64)
            if i == 0:
                nc.vector.tensor_mul(un[0:64, :], u[0:64, :], rb[:])
            else:
                tmp = sb.tile([64, QC], BF16, tag="untmp", bufs=2, name=f"ut{s}")
                nc.vector.tensor_mul(tmp[:], u[0:64, :], rb[:])
                nc.sync.dma_start(un[64:128, :], tmp[:])
        del U[s]
        UN[(qc, pair)] = un

    # ---------------- static schedule --------------------------------------
    # Fillers: (cols, closure); forced at their deadline round if not yet run.
    class Job:
        __slots__ = ("cols", "fn", "done")

        def __init__(self, cols, fn):
            self.cols, self.fn, self.done = cols, fn, False

        def run(self):
            if not self.done:
                self.done = True
                self.fn()

    def qk_job(kind, pr, idx):
        return Job(4096, lambda: emit_qk_group(kind, pr, idx))

    jobs = {}
    for kind, pr, idx in [
        ("q", 0, 1), ("q", 0, 2), ("q", 0, 3),
        ("k", 1, 0), ("k", 1, 1), ("k", 1, 2), ("k", 1, 3),
        ("q", 1, 0), ("q", 1, 1), ("q", 1, 2), ("q", 1, 3),
    ]:
        jobs[(kind, pr, idx)] = qk_job(kind, pr, idx)

    # EDF-ordered general filler queue with earliest-emission gates
    # (rounds before which the needed DMA has not landed yet).
    fq = deque(
        [
            (8, jobs[("q", 0, 1)]),
            (9, jobs[("k", 1, 0)]),
            (10, jobs[("q", 1, 0)]),
            (10, jobs[("q", 0, 2)]),
            (10, jobs[("k", 1, 1)]),
            (11, jobs[("k", 1, 2)]),
            (11, jobs[("k", 1, 3)]),
            (12, jobs[("q", 0, 3)]),
            (13, jobs[("q", 1, 1)]),
            (13, jobs[("q", 1, 2)]),
            (14, jobs[("q", 1, 3)]),
        ]
    )

    # mand_pre: tiles this round's scores read -> must be emitted first.
    jobs[("k", 0, 2)] = qk_job("k", 0, 2)
    jobs[("k", 0, 3)] = qk_job("k", 0, 3)
    mand_pre = defaultdict(list)
    mand_pre[2].append(jobs[("k", 0, 2)])
    mand_pre[4].append(jobs[("k", 0, 3)])
    mand_pre[8].append(jobs[("q", 0, 1)])
    mand_pre[16].append(jobs[("q", 0, 2)])
    mand_pre[24].append(jobs[("q", 0, 3)])
    mand_pre[32].append(jobs[("k", 1, 0)])
    mand_pre[32].append(jobs[("q", 1, 0)])
    mand_pre[34].append(jobs[("k", 1, 1)])
    mand_pre[36].append(jobs[("k", 1, 2)])
    mand_pre[38].append(jobs[("k", 1, 3)])
    mand_pre[40].append(jobs[("q", 1, 1)])
    mand_pre[48].append(jobs[("q", 1, 2)])
    mand_pre[56].append(jobs[("q", 1, 3)])

    # mand_post: V just-in-time (consumed by PV one/two rounds later).
    mand_post = defaultdict(list)
    for r in range(7):  # pair0 kt2..15
        mand_post[r].extend(
            Job(1024, (lambda p, k: (lambda: emit_v_group(p, k)))(0, kt))
            for kt in (2 * r + 2, 2 * r + 3)
        )
    for j, r in enumerate(range(24, 32)):  # pair1 kt0..15
        mand_post[r].extend(
            Job(1024, (lambda p, k: (lambda: emit_v_group(p, k)))(1, kt))
            for kt in (2 * j, 2 * j + 1)
        )

    # ---------------- preamble ---------------------------------------------
    emit_qk_group("k", 0, 0)
    emit_qk_group("k", 0, 1)
    emit_qk_group("q", 0, 0)
    mand_post[0].extend(
        Job(1024, (lambda p, k: (lambda: emit_v_group(p, k)))(0, kt))
        for kt in (0, 1)
    )

    # ---------------- main pipeline ----------------------------------------
    BUDGET = 2200
    for r in range(NR):
        for job in mand_pre[r]:
            job.run()
        emit_scores(r)
        if r > 0:
            emit_pv(r - 1)
        if r % 8 == 0 and r > 0:
            s = r // 8 - 1
            emit_normalize(s)
            pair, qc = divmod(s, 4)
            if pair == 1:
                pending.extend((qc, qi, ec) for qi in range(4) for ec in range(2))
        for job in mand_post[r]:
            job.run()
        budget = BUDGET
        while budget > 0:
            while fq and fq[0][1].done:
                fq.popleft()
            if fq and fq[0][0] <= r:
                _, job = fq.popleft()
                budget -= job.cols
                job.run()
            elif pending:
                emit_outproj_unit()
                budget -= 1024
            else:
                break

    # ---------------- drain -------------------------------------------------
    emit_pv(NR - 1)
    emit_normalize(7)
    pending.extend((3, qi, ec) for qi in range(4) for ec in range(2))
    for _, job in fq:
        job.run()
    tags = ["pj", "u", "u", "u"]
    i = 0
    while pending:
        emit_outproj_unit(tag=tags[i % 4])
        i += 1


def build_program():
    nc = bacc.Bacc(
        "TRN2", target_bir_lowering=False, debug=False, num_devices=NCORES
    )
    io = {
        "xq": nc.dram_tensor("xq", [NQC, P, CD, QC], BF16, kind="ExternalInput").ap(),
        "xk": nc.dram_tensor("xk", [NQC, P, CD, QC], BF16, kind="ExternalInput").ap(),
        "xv": nc.dram_tensor("xv", [NQC, P, CD, QC], BF16, kind="ExternalInput").ap(),
        "wq": nc.dram_tensor("wq", [2, P, CD, P], BF16, kind="ExternalInput").ap(),
        "wk": nc.dram_tensor("wk", [2, P, CD, P], BF16, kind="ExternalInput").ap(),
        "wv": nc.dram_tensor("wv", [2, P, CD, P], BF16, kind="ExternalInput").ap(),
        "wo": nc.dram_tensor("wo", [2, P, D], BF16, kind="ExternalInput").ap(),
        "bq": nc.dram_tensor("bq", [P, 2], F32, kind="ExternalInput").ap(),
        "ones2": nc.dram_tensor("ones2", [P, 2], BF16, kind="ExternalInput").ap(),
        "y": nc.dram_tensor("y", [S, D], BF16, kind="ExternalOutput").ap(),
    }
    with tile.TileContext(nc) as tc:
        with ExitStack() as ctx:
            _body(ctx, tc, io)
    nc.compile()
    return nc


_CACHE = {}


def _get_program():
    if "nc" not in _CACHE:
        _CACHE["nc"] = build_program()
    return _CACHE["nc"]


def make_in_maps(inputs):
    q = np.asarray(inputs["query"], np.float32)
    k = np.asarray(inputs["key"], np.float32)
    v = np.asarray(inputs["value"], np.float32)
    W_q = np.asarray(inputs["W_q"], np.float32)
    W_k = np.asarray(inputs["W_k"], np.float32)
    W_v = np.asarray(inputs["W_v"], np.float32)
    W_o = np.asarray(inputs["W_o"], np.float32)
    b_q = np.asarray(inputs["b_q"], np.float32)

    bf = ml_dtypes.bfloat16

    def xblocks(x):  # [S, D] activations -> [blk, p, c, s] with x.T blocked
        xt = np.ascontiguousarray(x.T).astype(bf)  # [D, S]
        return np.ascontiguousarray(
            xt.reshape(CD, P, NQC, QC).transpose(2, 1, 0, 3)
        )

    def wblocks(w_sl):  # [D, 256] (= W[sl].T) -> [pr, p, c, d]
        return np.ascontiguousarray(
            w_sl.reshape(CD, P, 2, P).transpose(2, 1, 0, 3).astype(bf)
        )

    xq = [xblocks(q[b]) for b in range(B)]
    xk = [xblocks(k[b]) for b in range(B)]
    xv = [xblocks(v[b]) for b in range(B)]

    in_maps = []
    for core in range(NCORES):
        b, g = divmod(core, NG)
        sl = slice(g * DG, (g + 1) * DG)
        in_maps.append(
            {
                "xq": xq[b],
                "xk": xk[b],
                "xv": xv[b],
                "wq": wblocks(W_q[sl, :].T),
                "wk": wblocks(W_k[sl, :].T),
                "wv": wblocks(W_v[sl, :].T),
                "wo": np.ascontiguousarray(
                    W_o[:, sl].T.reshape(2, P, D).astype(bf)
                ),
                "bq": np.ascontiguousarray(b_q[sl].reshape(2, P).T),
                "ones2": np.ones((P, 2), bf),
            }
        )
    return in_maps


def kernel(**inputs):
    from concourse.bass_utils import run_bass_kernel_spmd

    nc = _get_program()
    in_maps = make_in_maps(inputs)
    trace = bool(int(os.environ.get("MHA_TRACE", "0")))
    res = run_bass_kernel_spmd(nc, in_maps, list(range(NCORES)), trace=trace)
    _CACHE["last_results"] = res

    W_o = np.asarray(inputs["W_o"], np.float64)
    b_o = np.asarray(inputs["b_o"], np.float64)
    b_v = np.asarray(inputs["b_v"], np.float64)
    out = np.zeros((B, S, D), np.float32)
    for core in range(NCORES):
        b = core // NG
        out[b] += res.results[core]["y"].astype(np.float32)
    # b_v and b_o commute with the attention average / output projection.
    out += (b_o + b_v @ W_o.T).astype(np.float32)[None, None, :]
    return out


# revision 16
# speedup vs baseline: 1.1018x; 1.0002x over previous
"""Multi-head attention (B=2, S=2048, D=1024, H=16) on 8 Trainium2 cores.

Sharding: core = 4*b + g  (b = batch 0..1, g = head-group 0..3, 4 heads each).
Heads are processed in pairs; pair p covers the group's d-dims [128p, 128p+128).

Schedule: the scalar engine's exp stream (128 activations of [128,1024],
~172us) is the critical resource.  A short DMA-led preamble projects only
K(pair0), Q(pair0,qc0), V(kt0,1); then 64 pipelined rounds (one per
(pair, qc, k-group)) keep ACT continuously busy:

  round r: [forced proj groups] scores(r) -> exp(r) -> PV(r-1)
           [normalize at sweep boundaries] [filler: proj / out-proj]

All other projections (K pair1, remaining Q, V) and the output projection
run as PE filler inside the rounds' slack so the tensor engine never idles
(and stays at full DVFS pstate).  Sweep order is pair-major so pair1's
weights/projections have 4 sweeps of slack to materialize.

Exactness notes: b_k only shifts each softmax row uniformly -> dropped.
b_v and b_o commute with softmax-average -> folded into the host reduce.
b_q is applied on-device (fused into the Q PSUM->SBUF copy).
"""

import os
from collections import defaultdict, deque
from contextlib import ExitStack

import ml_dtypes
import numpy as np

import concourse.bass as bass
import concourse.tile as tile
from concourse import bacc, mybir

B, S, D = 2, 2048, 1024
H, DH = 16, 64
NCORES = 8
NG = 4                  # head-group shards
DG = D // NG            # 256 dims per head-group (4 heads, 2 pairs)
P = 128
QC = 512                # q-chunk width
NQC = S // QC           # 4
NKT = S // P            # 16 k-tiles of 128
CD = D // P             # 8 contraction tiles for the projections
NR = 64                 # pipeline rounds: 2 pairs x 4 qc x 8 k-groups
F32 = mybir.dt.float32
BF16 = mybir.dt.bfloat16
AF = mybir.ActivationFunctionType
SCALE = 1.0 / float(np.sqrt(D))


def _body(ctx: ExitStack, tc: "tile.TileContext", io: dict):
    nc = tc.nc
    ctx.enter_context(nc.allow_low_precision(reason="bf16 matmul pipeline"))
    sb = ctx.enter_context(tc.tile_pool(name="sb", bufs=1))
    ps = ctx.enter_context(tc.tile_pool(name="ps", bufs=1, space="PSUM"))

    # ---------------- DMA: inputs stream in consumption order --------------
    xk_sb, xq_sb, xv_sb = {}, {}, {}
    w_sb = {}

    def dma_x(dst_map, key, idx):
        t = sb.tile([P, CD, QC], BF16, tag="x", bufs=12, name=f"{key}{idx}")
        # two halves so projection groups can start on the first half
        nc.sync.dma_start(t[:, 0:4, :], io[key][idx, :, 0:4, :])
        nc.sync.dma_start(t[:, 4:8, :], io[key][idx, :, 4:8, :])
        dst_map[idx] = t

    def dma_w(kind, pr):
        t = sb.tile([P, CD, P], BF16, tag="w", bufs=6, name=f"w{kind}{pr}")
        nc.sync.dma_start(t[:], io[f"w{kind}"][pr])
        w_sb[(kind, pr)] = t

    dma_w("k", 0)
    dma_x(xk_sb, "xk", 0)
    dma_w("q", 0)
    bq = sb.tile([P, 2], F32, tag="bq", bufs=1, name="bq")
    nc.sync.dma_start(bq[:], io["bq"])
    ones2 = sb.tile([P, 2], BF16, tag="ones2", bufs=1, name="ones2")
    nc.sync.dma_start(ones2[:], io["ones2"])
    dma_x(xk_sb, "xk", 1)
    dma_x(xq_sb, "xq", 0)
    dma_w("v", 0)
    dma_x(xv_sb, "xv", 0)
    dma_x(xv_sb, "xv", 1)
    dma_x(xq_sb, "xq", 1)
    dma_x(xk_sb, "xk", 2)
    dma_x(xv_sb, "xv", 2)
    dma_x(xk_sb, "xk", 3)
    dma_x(xv_sb, "xv", 3)
    dma_w("k", 1)
    dma_w("q", 1)
    dma_x(xq_sb, "xq", 2)
    dma_w("v", 1)
    dma_x(xq_sb, "xq", 3)
    woT = []
    for pr in range(2):
        t = sb.tile([P, D], BF16, tag="wo", bufs=2, name=f"woT{pr}")
        nc.sync.dma_start(t[:], io["wo"][pr])
        woT.append(t)

    # ---------------- projection / out-proj emitters -----------------------
    KT, QT, V, UN, YSB = {}, {}, {}, {}, {}

    def emit_qk_group(kind, pr, idx):
        w = w_sb[(kind, pr)]
        x = (xk_sb if kind == "k" else xq_sb)[idx]
        pg = ps.tile([P, QC], F32, tag="pj", bufs=1, name=f"pg{kind}{pr}{idx}")
        for c in range(CD):
            nc.tensor.matmul(
                pg[:], w[:, c, :], x[:, c, :], start=(c == 0), stop=(c == CD - 1)
            )
        t = sb.tile([P, QC], BF16, tag=f"{kind}t", bufs=8, name=f"{kind}T{pr}_{idx}")
        if kind == "q":
            nc.vector.tensor_scalar_add(t[:], pg[:], bq[:, pr : pr + 1])
            QT[(pr, idx)] = t
        else:
            nc.vector.tensor_copy(t[:], pg[:])
            KT[(pr, idx)] = t

    def emit_v_group(pair, kt):
        sc, off = divmod(kt, 4)
        x = xv_sb[sc]
        pg = ps.tile([P, P], F32, tag="pj", bufs=1, name=f"pgv{pair}{kt}")
        for c in range(CD):
            nc.tensor.matmul(
                pg[:],
                x[:, c, off * P : (off + 1) * P],
                w_sb[("v", pair)][:, c, :],
                start=(c == 0),
                stop=(c == CD - 1),
            )
        vt = sb.tile([P, 2, DH + 1], BF16, tag="v", bufs=32, name=f"V{pair}_{kt}")
        nc.vector.tensor_copy(vt[:, :, 0:DH], pg[:].rearrange("p (i d) -> p i d", i=2))
        nc.vector.tensor_copy(vt[:, :, DH : DH + 1], ones2[:, :, None])
        V[(pair, kt)] = vt

    pending = deque()
    half_pending = deque()

    def emit_outproj_half(tag="pj"):
        # pair0 half of a qc3 unit; bf16 partial parked in SBUF
        qi, ec = half_pending.popleft()
        qt = 12 + qi
        if ec == 0:
            YSB[qt] = sb.tile([P, D], BF16, tag="y3", bufs=4, name=f"YP{qt}")
        yp = ps.tile(
            [P, QC], F32, tag=tag, bufs=(1 if tag == "pj" else 3), name=f"yh{qt}_{ec}"
        )
        nc.tensor.matmul(
            yp[:],
            UN[(3, 0)][:, qi * P : (qi + 1) * P],
            woT[0][:, ec * QC : (ec + 1) * QC],
            start=True,
            stop=True,
        )
        nc.vector.tensor_copy(YSB[qt][:, ec * QC : (ec + 1) * QC], yp[:])

    def emit_outproj_unit(tag="pj"):
        qc, qi, ec = pending.popleft()
        qt = qc * 4 + qi
        if ec == 0:
            YSB[qt] = sb.tile([P, D], BF16, tag="y", bufs=4, name=f"Y{qt}")
        ysb = YSB[qt]
        yp = ps.tile([P, QC], F32, tag=tag, bufs=(1 if tag == "pj" else 3), name=f"yp{qt}_{ec}")
        for pr in range(2):
            nc.tensor.matmul(
                yp[:],
                UN[(qc, pr)][:, qi * P : (qi + 1) * P],
                woT[pr][:, ec * QC : (ec + 1) * QC],
                start=(pr == 0),
                stop=(pr == 1),
            )
        nc.vector.tensor_copy(ysb[:, ec * QC : (ec + 1) * QC], yp[:])
        if ec == 1:
            nc.sync.dma_start(io["y"][qt * P : (qt + 1) * P, :], ysb[:])

    # ---------------- attention round emitters -----------------------------
    PTs, U = {}, {}

    def sweep_of(r):
        s = r // 8
        pair, qc = divmod(s, 4)
        return s, pair, qc, r % 8

    def emit_scores(r):
        _, pair, qc, kg = sweep_of(r)
        for i in (0, 1):
            lo = 64 * i
            st = ps.tile([P, 2, QC], F32, tag="st", bufs=2, name=f"st{r}_{i}")
            for kk in (0, 1):
                kt = kg * 2 + kk
                sc, off = divmod(kt, 4)
                nc.tensor.matmul(
                    st[:, kk, :],
                    KT[(pair, sc)][lo : lo + 64, off * P : (off + 1) * P],
                    QT[(pair, qc)][lo : lo + 64, :],
                    start=True,
                    stop=True,
                    tile_position=(lo, 0),
                )
            pt = sb.tile([P, 2, QC], BF16, tag="pt", bufs=4, name=f"pt{r}_{i}")
            nc.scalar.activation(
                pt[:].rearrange("p a b -> p (a b)"),
                st[:].rearrange("p a b -> p (a b)"),
                AF.Exp,
                scale=SCALE,
            )
            PTs[(r, i)] = pt

    def emit_pv(r):
        s, pair, qc, kg = sweep_of(r)
        if kg == 0:
            U[s] = [
                ps.tile([P, QC], F32, tag="u", bufs=3, name=f"U{s}_{i}") for i in (0, 1)
            ]
        for i in (0, 1):
            pt = PTs.pop((r, i))
            for kk in (0, 1):
                kt = kg * 2 + kk
                nc.tensor.matmul(
                    U[s][i][0:65, :],
                    V[(pair, kt)][:, i, :],
                    pt[:, kk, :],
                    start=(kg == 0 and kk == 0),
                    stop=(kg == 7 and kk == 1),
                )

    def emit_normalize(s):
        pair, qc = divmod(s, 4)
        un = sb.tile([P, QC], BF16, tag="un", bufs=8, name=f"UN{qc}_{pair}")
        for i in (0, 1):
            u = U[s][i]
            zr = sb.tile([65, QC], F32, tag="zr", bufs=2, name=f"zr{s}_{i}")
            nc.vector.tensor_copy(zr[64:65, :], u[64:65, :])
            z = sb.tile([1, QC], F32, tag="z", bufs=2, name=f"z{s}_{i}")
            nc.gpsimd.dma_start(z[:], zr[64:65, :])
            rz = sb.tile([1, QC], F32, tag="rz", bufs=2, name=f"rz{s}_{i}")
            nc.vector.reciprocal(rz[:], z[:])
            rb = sb.tile([64, QC], F32, tag="rb", bufs=2, name=f"rb{s}_{i}")
            nc.gpsimd.partition_broadcast(rb[:], rz[:], channels=64)
            if i == 0:
                nc.vector.tensor_mul(un[0:64, :], u[0:64, :], rb[:])
            else:
                tmp = sb.tile([64, QC], BF16, tag="untmp", bufs=2, name=f"ut{s}")
                nc.vector.tensor_mul(tmp[:], u[0:64, :], rb[:])
                nc.gpsimd.dma_start(un[64:128, :], tmp[:])
        del U[s]
        UN[(qc, pair)] = un
        if pair == 0 and qc == 3:
            half_pending.extend((qi, ec) for qi in range(4) for ec in range(2))

    # ---------------- static schedule --------------------------------------
    # Fillers: (cols, closure); forced at their deadline round if not yet run.
    class Job:
        __slots__ = ("cols", "fn", "done")

        def __init__(self, cols, fn):
            self.cols, self.fn, self.done = cols, fn, False

        def run(self):
            if not self.done:
                self.done = True
                self.fn()

    def qk_job(kind, pr, idx):
        return Job(4096, lambda: emit_qk_group(kind, pr, idx))

    jobs = {}
    for kind, pr, idx in [
        ("q", 0, 1), ("q", 0, 2), ("q", 0, 3),
        ("k", 1, 0), ("k", 1, 1), ("k", 1, 2), ("k", 1, 3),
        ("q", 1, 0), ("q", 1, 1), ("q", 1, 2), ("q", 1, 3),
    ]:
        jobs[(kind, pr, idx)] = qk_job(kind, pr, idx)

    # EDF-ordered general filler queue with earliest-emission gates
    # (rounds before which the needed DMA has not landed yet).
    fq = deque(
        [
            (8, jobs[("q", 0, 1)]),
            (9, jobs[("k", 1, 0)]),
            (10, jobs[("q", 1, 0)]),
            (10, jobs[("q", 0, 2)]),
            (10, jobs[("k", 1, 1)]),
            (11, jobs[("k", 1, 2)]),
            (11, jobs[("k", 1, 3)]),
            (12, jobs[("q", 0, 3)]),
            (13, jobs[("q", 1, 1)]),
            (13, jobs[("q", 1, 2)]),
            (14, jobs[("q", 1, 3)]),
        ]
        + [(33 + j, Job(512, lambda: emit_outproj_half())) for j in range(8)]
    )

    # mand_pre: tiles this round's scores read -> must be emitted first.
    jobs[("k", 0, 2)] = qk_job("k", 0, 2)
    jobs[("k", 0, 3)] = qk_job("k", 0, 3)
    mand_pre = defaultdict(list)
    mand_pre[2].append(jobs[("k", 0, 2)])
    mand_pre[4].append(jobs[("k", 0, 3)])
    mand_pre[8].append(jobs[("q", 0, 1)])
    mand_pre[16].append(jobs[("q", 0, 2)])
    mand_pre[24].append(jobs[("q", 0, 3)])
    mand_pre[32].append(jobs[("k", 1, 0)])
    mand_pre[32].append(jobs[("q", 1, 0)])
    mand_pre[34].append(jobs[("k", 1, 1)])
    mand_pre[36].append(jobs[("k", 1, 2)])
    mand_pre[38].append(jobs[("k", 1, 3)])
    mand_pre[40].append(jobs[("q", 1, 1)])
    mand_pre[48].append(jobs[("q", 1, 2)])
    mand_pre[56].append(jobs[("q", 1, 3)])

    # mand_post: V just-in-time (consumed by PV one/two rounds later).
    mand_post = defaultdict(list)
    for r in range(7):  # pair0 kt2..15
        mand_post[r].extend(
            Job(1024, (lambda p, k: (lambda: emit_v_group(p, k)))(0, kt))
            for kt in (2 * r + 2, 2 * r + 3)
        )
    for j, r in enumerate(range(24, 32)):  # pair1 kt0..15
        mand_post[r].extend(
            Job(1024, (lambda p, k: (lambda: emit_v_group(p, k)))(1, kt))
            for kt in (2 * j, 2 * j + 1)
        )

    # ---------------- preamble ---------------------------------------------
    emit_qk_group("k", 0, 0)
    emit_qk_group("k", 0, 1)
    emit_qk_group("q", 0, 0)
    mand_post[0].extend(
        Job(1024, (lambda p, k: (lambda: emit_v_group(p, k)))(0, kt))
        for kt in (0, 1)
    )

    # ---------------- main pipeline ----------------------------------------
    BUDGET = 2200
    for r in range(NR):
        for job in mand_pre[r]:
            job.run()
        emit_scores(r)
        if r > 0:
            emit_pv(r - 1)
        if r % 8 == 0 and r > 0:
            s = r // 8 - 1
            emit_normalize(s)
            pair, qc = divmod(s, 4)
            if pair == 1:
                pending.extend((qc, qi, ec) for qi in range(4) for ec in range(2))
        for job in mand_post[r]:
            job.run()
        budget = BUDGET
        while budget > 0:
            while fq and fq[0][1].done:
                fq.popleft()
            if fq and fq[0][0] <= r:
                _, job = fq.popleft()
                budget -= job.cols
                job.run()
            elif pending:
                emit_outproj_unit()
                budget -= 1024
            else:
                break

    # ---------------- drain -------------------------------------------------
    emit_pv(NR - 1)
    emit_normalize(7)
    for _, job in fq:
        job.run()
    tags = ["pj", "u", "u", "u"]
    i = 0
    while pending:
        emit_outproj_unit(tag=tags[i % 4])
        i += 1
    while half_pending:
        emit_outproj_half(tag=tags[i % 4])
        i += 1
    # qc3 completions: all pair1 matmuls first (4 PSUM slots in flight),
    # adds/DMAs drain on DVE/sync concurrently.
    comp = []
    for n, (qi, ec) in enumerate([(a, b) for a in range(4) for b in range(2)]):
        tag = tags[n % 4]
        yp = ps.tile(
            [P, QC], F32, tag=tag, bufs=(1 if tag == "pj" else 3), name=f"yc{qi}_{ec}"
        )
        nc.tensor.matmul(
            yp[:],
            UN[(3, 1)][:, qi * P : (qi + 1) * P],
            woT[1][:, ec * QC : (ec + 1) * QC],
            start=True,
            stop=True,
        )
        comp.append((qi, ec, yp))
    for qi, ec, yp in comp:
        qt = 12 + qi
        ysb = YSB[qt]
        nc.vector.tensor_add(
            ysb[:, ec * QC : (ec + 1) * QC], yp[:], ysb[:, ec * QC : (ec + 1) * QC]
        )
        nc.sync.dma_start(
            io["y"][qt * P : (qt + 1) * P, ec * QC : (ec + 1) * QC],
            ysb[:, ec * QC : (ec + 1) * QC],
        )


def build_program():
    nc = bacc.Bacc(
        "TRN2", target_bir_lowering=False, debug=False, num_devices=NCORES
    )
    io = {
        "xq": nc.dram_tensor("xq", [NQC, P, CD, QC], BF16, kind="ExternalInput").ap(),
        "xk": nc.dram_tensor("xk", [NQC, P, CD, QC], BF16, kind="ExternalInput").ap(),
        "xv": nc.dram_tensor("xv", [NQC, P, CD, QC], BF16, kind="ExternalInput").ap(),
        "wq": nc.dram_tensor("wq", [2, P, CD, P], BF16, kind="ExternalInput").ap(),
        "wk": nc.dram_tensor("wk", [2, P, CD, P], BF16, kind="ExternalInput").ap(),
        "wv": nc.dram_tensor("wv", [2, P, CD, P], BF16, kind="ExternalInput").ap(),
        "wo": nc.dram_tensor("wo", [2, P, D], BF16, kind="ExternalInput").ap(),
        "bq": nc.dram_tensor("bq", [P, 2], F32, kind="ExternalInput").ap(),
        "ones2": nc.dram_tensor("ones2", [P, 2], BF16, kind="ExternalInput").ap(),
        "y": nc.dram_tensor("y", [S, D], BF16, kind="ExternalOutput").ap(),
    }
    with tile.TileContext(nc) as tc:
        with ExitStack() as ctx:
            _body(ctx, tc, io)
    nc.compile()
    return nc


_CACHE = {}


def _get_program():
    if "nc" not in _CACHE:
        _CACHE["nc"] = build_program()
    return _CACHE["nc"]


def make_in_maps(inputs):
    q = np.asarray(inputs["query"], np.float32)
    k = np.asarray(inputs["key"], np.float32)
    v = np.asarray(inputs["value"], np.float32)
    W_q = np.asarray(inputs["W_q"], np.float32)
    W_k = np.asarray(inputs["W_k"], np.float32)
    W_v = np.asarray(inputs["W_v"], np.float32)
    W_o = np.asarray(inputs["W_o"], np.float32)
    b_q = np.asarray(inputs["b_q"], np.float32)

    bf = ml_dtypes.bfloat16

    def xblocks(x):  # [S, D] activations -> [blk, p, c, s] with x.T blocked
        xt = np.ascontiguousarray(x.T).astype(bf)  # [D, S]
        return np.ascontiguousarray(
            xt.reshape(CD, P, NQC, QC).transpose(2, 1, 0, 3)
        )

    def wblocks(w_sl):  # [D, 256] (= W[sl].T) -> [pr, p, c, d]
        return np.ascontiguousarray(
            w_sl.reshape(CD, P, 2, P).transpose(2, 1, 0, 3).astype(bf)
        )

    xq = [xblocks(q[b]) for b in range(B)]
    xk = [xblocks(k[b]) for b in range(B)]
    xv = [xblocks(v[b]) for b in range(B)]

    in_maps = []
    for core in range(NCORES):
        b, g = divmod(core, NG)
        sl = slice(g * DG, (g + 1) * DG)
        in_maps.append(
            {
                "xq": xq[b],
                "xk": xk[b],
                "xv": xv[b],
                "wq": wblocks(W_q[sl, :].T),
                "wk": wblocks(W_k[sl, :].T),
                "wv": wblocks(W_v[sl, :].T),
                "wo": np.ascontiguousarray(
                    W_o[:, sl].T.reshape(2, P, D).astype(bf)
                ),
                "bq": np.ascontiguousarray(b_q[sl].reshape(2, P).T),
                "ones2": np.ones((P, 2), bf),
            }
        )
    return in_maps


def kernel(**inputs):
    from concourse.bass_utils import run_bass_kernel_spmd

    nc = _get_program()
    in_maps = make_in_maps(inputs)
    trace = bool(int(os.environ.get("MHA_TRACE", "0")))
    res = run_bass_kernel_spmd(nc, in_maps, list(range(NCORES)), trace=trace)
    _CACHE["last_results"] = res

    W_o = np.asarray(inputs["W_o"], np.float64)
    b_o = np.asarray(inputs["b_o"], np.float64)
    b_v = np.asarray(inputs["b_v"], np.float64)
    out = np.zeros((B, S, D), np.float32)
    for core in range(NCORES):
        b = core // NG
        out[b] += res.results[core]["y"].astype(np.float32)
    # b_v and b_o commute with the attention average / output projection.
    out += (b_o + b_v @ W_o.T).astype(np.float32)[None, None, :]
    return out


# revision 18
# speedup vs baseline: 1.1185x; 1.0152x over previous
"""Multi-head attention (B=2, S=2048, D=1024, H=16) on 8 Trainium2 cores.

Sharding: core = 4*b + g  (b = batch 0..1, g = head-group 0..3, 4 heads each).
Heads are processed in pairs; pair p covers the group's d-dims [128p, 128p+128).

Schedule: the scalar engine's exp stream (128 activations of [128,1024],
~172us) is the critical resource.  A short DMA-led preamble projects only
K(pair0), Q(pair0,qc0), V(kt0,1); then 64 pipelined rounds (one per
(pair, qc, k-group)) keep ACT continuously busy:

  round r: [forced proj groups] scores(r) -> exp(r) -> PV(r-1)
           [normalize at sweep boundaries] [filler: proj / out-proj]

All other projections (K pair1, remaining Q, V) and the output projection
run as PE filler inside the rounds' slack so the tensor engine never idles
(and stays at full DVFS pstate).  Sweep order is pair-major so pair1's
weights/projections have 4 sweeps of slack to materialize.

Exactness notes: b_k only shifts each softmax row uniformly -> dropped.
b_v and b_o commute with softmax-average -> folded into the host reduce.
b_q is applied on-device (fused into the Q PSUM->SBUF copy).
"""

import os
from collections import defaultdict, deque
from contextlib import ExitStack

import ml_dtypes
import numpy as np

import concourse.bass as bass
import concourse.tile as tile
from concourse import bacc, mybir

B, S, D = 2, 2048, 1024
H, DH = 16, 64
NCORES = 8
NG = 4                  # head-group shards
DG = D // NG            # 256 dims per head-group (4 heads, 2 pairs)
P = 128
QC = 512                # q-chunk width
NQC = S // QC           # 4
NKT = S // P            # 16 k-tiles of 128
CD = D // P             # 8 contraction tiles for the projections
NR = 64                 # pipeline rounds: 2 pairs x 4 qc x 8 k-groups
F32 = mybir.dt.float32
BF16 = mybir.dt.bfloat16
AF = mybir.ActivationFunctionType
SCALE = 1.0 / float(np.sqrt(D))


def _body(ctx: ExitStack, tc: "tile.TileContext", io: dict):
    nc = tc.nc
    ctx.enter_context(nc.allow_low_precision(reason="bf16 matmul pipeline"))
    sb = ctx.enter_context(tc.tile_pool(name="sb", bufs=1))
    ps = ctx.enter_context(tc.tile_pool(name="ps", bufs=1, space="PSUM"))

    # ---------------- DMA: inputs stream in consumption order --------------
    xk_sb, xq_sb, xv_sb = {}, {}, {}
    w_sb = {}

    def dma_x(dst_map, key, idx):
        t = sb.tile([P, CD, QC], BF16, tag="x", bufs=12, name=f"{key}{idx}")
        # two halves so projection groups can start on the first half
        nc.sync.dma_start(t[:, 0:4, :], io[key][idx, :, 0:4, :])
        nc.sync.dma_start(t[:, 4:8, :], io[key][idx, :, 4:8, :])
        dst_map[idx] = t

    def dma_w(kind, pr):
        t = sb.tile([P, CD, P], BF16, tag="w", bufs=6, name=f"w{kind}{pr}")
        nc.sync.dma_start(t[:], io[f"w{kind}"][pr])
        w_sb[(kind, pr)] = t

    dma_w("k", 0)
    dma_x(xk_sb, "xk", 0)
    dma_w("q", 0)
    bq = sb.tile([P, 2], F32, tag="bq", bufs=1, name="bq")
    nc.sync.dma_start(bq[:], io["bq"])
    ones2 = sb.tile([P, 2], BF16, tag="ones2", bufs=1, name="ones2")
    nc.sync.dma_start(ones2[:], io["ones2"])
    dma_x(xq_sb, "xq", 0)
    dma_x(xk_sb, "xk", 1)
    dma_w("v", 0)
    dma_x(xv_sb, "xv", 0)
    dma_x(xv_sb, "xv", 1)
    dma_x(xq_sb, "xq", 1)
    dma_x(xk_sb, "xk", 2)
    dma_x(xv_sb, "xv", 2)
    dma_x(xk_sb, "xk", 3)
    dma_x(xv_sb, "xv", 3)
    dma_w("k", 1)
    dma_w("q", 1)
    dma_x(xq_sb, "xq", 2)
    dma_w("v", 1)
    dma_x(xq_sb, "xq", 3)
    woT = []
    for pr in range(2):
        t = sb.tile([P, D], BF16, tag="wo", bufs=2, name=f"woT{pr}")
        nc.sync.dma_start(t[:], io["wo"][pr])
        woT.append(t)

    # ---------------- projection / out-proj emitters -----------------------
    KT, QT, V, UN, YSB = {}, {}, {}, {}, {}

    def emit_qk_group(kind, pr, idx):
        w = w_sb[(kind, pr)]
        x = (xk_sb if kind == "k" else xq_sb)[idx]
        pg = ps.tile([P, QC], F32, tag="pj", bufs=1, name=f"pg{kind}{pr}{idx}")
        for c in range(CD):
            nc.tensor.matmul(
                pg[:], w[:, c, :], x[:, c, :], start=(c == 0), stop=(c == CD - 1)
            )
        t = sb.tile([P, QC], BF16, tag=f"{kind}t", bufs=8, name=f"{kind}T{pr}_{idx}")
        if kind == "q":
            nc.vector.tensor_scalar_add(t[:], pg[:], bq[:, pr : pr + 1])
            QT[(pr, idx)] = t
        else:
            nc.vector.tensor_copy(t[:], pg[:])
            KT[(pr, idx)] = t

    def emit_v_group(pair, kt):
        sc, off = divmod(kt, 4)
        x = xv_sb[sc]
        pg = ps.tile([P, P], F32, tag="pj", bufs=1, name=f"pgv{pair}{kt}")
        for c in range(CD):
            nc.tensor.matmul(
                pg[:],
                x[:, c, off * P : (off + 1) * P],
                w_sb[("v", pair)][:, c, :],
                start=(c == 0),
                stop=(c == CD - 1),
            )
        vt = sb.tile([P, 2, DH + 1], BF16, tag="v", bufs=32, name=f"V{pair}_{kt}")
        nc.vector.tensor_copy(vt[:, :, 0:DH], pg[:].rearrange("p (i d) -> p i d", i=2))
        nc.vector.tensor_copy(vt[:, :, DH : DH + 1], ones2[:, :, None])
        V[(pair, kt)] = vt

    pending = deque()
    half_pending = deque()

    def emit_outproj_half(tag="pj"):
        # pair0 half of a qc3 unit; bf16 partial parked in SBUF
        qi, ec = half_pending.popleft()
        qt = 12 + qi
        if ec == 0:
            YSB[qt] = sb.tile([P, D], BF16, tag="y3", bufs=4, name=f"YP{qt}")
        yp = ps.tile(
            [P, QC], F32, tag=tag, bufs=(1 if tag == "pj" else 3), name=f"yh{qt}_{ec}"
        )
        nc.tensor.matmul(
            yp[:],
            UN[(3, 0)][:, qi * P : (qi + 1) * P],
            woT[0][:, ec * QC : (ec + 1) * QC],
            start=True,
            stop=True,
        )
        nc.vector.tensor_copy(YSB[qt][:, ec * QC : (ec + 1) * QC], yp[:])

    def emit_outproj_unit(tag="pj"):
        qc, qi, ec = pending.popleft()
        qt = qc * 4 + qi
        if ec == 0:
            YSB[qt] = sb.tile([P, D], BF16, tag="y", bufs=4, name=f"Y{qt}")
        ysb = YSB[qt]
        yp = ps.tile([P, QC], F32, tag=tag, bufs=(1 if tag == "pj" else 3), name=f"yp{qt}_{ec}")
        for pr in range(2):
            nc.tensor.matmul(
                yp[:],
                UN[(qc, pr)][:, qi * P : (qi + 1) * P],
                woT[pr][:, ec * QC : (ec + 1) * QC],
                start=(pr == 0),
                stop=(pr == 1),
            )
        nc.vector.tensor_copy(ysb[:, ec * QC : (ec + 1) * QC], yp[:])
        if ec == 1:
            nc.sync.dma_start(io["y"][qt * P : (qt + 1) * P, :], ysb[:])

    # ---------------- attention round emitters -----------------------------
    PTs, U = {}, {}

    def sweep_of(r):
        s = r // 8
        pair, qc = divmod(s, 4)
        return s, pair, qc, r % 8

    def emit_scores(r):
        _, pair, qc, kg = sweep_of(r)
        for i in (0, 1):
            lo = 64 * i
            st = ps.tile([P, 2, QC], F32, tag="st", bufs=2, name=f"st{r}_{i}")
            for kk in (0, 1):
                kt = kg * 2 + kk
                sc, off = divmod(kt, 4)
                nc.tensor.matmul(
                    st[:, kk, :],
                    KT[(pair, sc)][lo : lo + 64, off * P : (off + 1) * P],
                    QT[(pair, qc)][lo : lo + 64, :],
                    start=True,
                    stop=True,
                    tile_position=(lo, 0),
                )
            pt = sb.tile([P, 2, QC], BF16, tag="pt", bufs=4, name=f"pt{r}_{i}")
            nc.scalar.activation(
                pt[:].rearrange("p a b -> p (a b)"),
                st[:].rearrange("p a b -> p (a b)"),
                AF.Exp,
                scale=SCALE,
            )
            PTs[(r, i)] = pt

    def emit_pv(r):
        s, pair, qc, kg = sweep_of(r)
        if kg == 0:
            U[s] = [
                ps.tile([P, QC], F32, tag="u", bufs=3, name=f"U{s}_{i}") for i in (0, 1)
            ]
        for i in (0, 1):
            pt = PTs.pop((r, i))
            for kk in (0, 1):
                kt = kg * 2 + kk
                nc.tensor.matmul(
                    U[s][i][0:65, :],
                    V[(pair, kt)][:, i, :],
                    pt[:, kk, :],
                    start=(kg == 0 and kk == 0),
                    stop=(kg == 7 and kk == 1),
                )

    def emit_normalize(s):
        pair, qc = divmod(s, 4)
        un = sb.tile([P, QC], BF16, tag="un", bufs=8, name=f"UN{qc}_{pair}")
        for i in (0, 1):
            u = U[s][i]
            zr = sb.tile([65, QC], F32, tag="zr", bufs=2, name=f"zr{s}_{i}")
            nc.vector.tensor_copy(zr[64:65, :], u[64:65, :])
            z = sb.tile([1, QC], F32, tag="z", bufs=2, name=f"z{s}_{i}")
            nc.sync.dma_start(z[:], zr[64:65, :])
            rz = sb.tile([1, QC], F32, tag="rz", bufs=2, name=f"rz{s}_{i}")
            nc.vector.reciprocal(rz[:], z[:])
            rb = sb.tile([64, QC], F32, tag="rb", bufs=2, name=f"rb{s}_{i}")
            nc.gpsimd.partition_broadcast(rb[:], rz[:], channels=64)
            if i == 0:
                nc.vector.tensor_mul(un[0:64, :], u[0:64, :], rb[:])
            else:
                tmp = sb.tile([64, QC], BF16, tag="untmp", bufs=2, name=f"ut{s}")
                nc.vector.tensor_mul(tmp[:], u[0:64, :], rb[:])
                nc.sync.dma_start(un[64:128, :], tmp[:])
        del U[s]
        UN[(qc, pair)] = un
        if pair == 0 and qc == 3:
            half_pending.extend((qi, ec) for qi in range(4) for ec in range(2))

    # ---------------- static schedule --------------------------------------
    # Fillers: (cols, closure); forced at their deadline round if not yet run.
    class Job:
        __slots__ = ("cols", "fn", "done")

        def __init__(self, cols, fn):
            self.cols, self.fn, self.done = cols, fn, False

        def run(self):
            if not self.done:
                self.done = True
                self.fn()

    def qk_job(kind, pr, idx):
        return Job(4096, lambda: emit_qk_group(kind, pr, idx))

    jobs = {}
    for kind, pr, idx in [
        ("q", 0, 1), ("q", 0, 2), ("q", 0, 3),
        ("k", 1, 0), ("k", 1, 1), ("k", 1, 2), ("k", 1, 3),
        ("q", 1, 0), ("q", 1, 1), ("q", 1, 2), ("q", 1, 3),
    ]:
        jobs[(kind, pr, idx)] = qk_job(kind, pr, idx)

    # EDF-ordered general filler queue with earliest-emission gates
    # (rounds before which the needed DMA has not landed yet).
    fq = deque(
        [
            (8, jobs[("q", 0, 1)]),
            (9, jobs[("k", 1, 0)]),
            (10, jobs[("q", 1, 0)]),
            (10, jobs[("q", 0, 2)]),
            (10, jobs[("k", 1, 1)]),
            (11, jobs[("k", 1, 2)]),
            (11, jobs[("k", 1, 3)]),
            (12, jobs[("q", 0, 3)]),
            (13, jobs[("q", 1, 1)]),
            (13, jobs[("q", 1, 2)]),
            (14, jobs[("q", 1, 3)]),
        ]
        + [(33 + j, Job(512, lambda: emit_outproj_half())) for j in range(8)]
    )

    # mand_pre: tiles this round's scores read -> must be emitted first.
    jobs[("k", 0, 2)] = qk_job("k", 0, 2)
    jobs[("k", 0, 3)] = qk_job("k", 0, 3)
    mand_pre = defaultdict(list)
    mand_pre[2].append(jobs[("k", 0, 2)])
    mand_pre[4].append(jobs[("k", 0, 3)])
    mand_pre[8].append(jobs[("q", 0, 1)])
    mand_pre[16].append(jobs[("q", 0, 2)])
    mand_pre[24].append(jobs[("q", 0, 3)])
    mand_pre[32].append(jobs[("k", 1, 0)])
    mand_pre[32].append(jobs[("q", 1, 0)])
    mand_pre[34].append(jobs[("k", 1, 1)])
    mand_pre[36].append(jobs[("k", 1, 2)])
    mand_pre[38].append(jobs[("k", 1, 3)])
    mand_pre[40].append(jobs[("q", 1, 1)])
    mand_pre[48].append(jobs[("q", 1, 2)])
    mand_pre[56].append(jobs[("q", 1, 3)])

    # mand_post: V just-in-time (consumed by PV one/two rounds later).
    mand_post = defaultdict(list)
    for r in range(7):  # pair0 kt2..15
        mand_post[r].extend(
            Job(1024, (lambda p, k: (lambda: emit_v_group(p, k)))(0, kt))
            for kt in (2 * r + 2, 2 * r + 3)
        )
    for j, r in enumerate(range(24, 32)):  # pair1 kt0..15
        mand_post[r].extend(
            Job(1024, (lambda p, k: (lambda: emit_v_group(p, k)))(1, kt))
            for kt in (2 * j, 2 * j + 1)
        )

    # ---------------- preamble ---------------------------------------------
    emit_qk_group("k", 0, 0)
    emit_qk_group("q", 0, 0)
    emit_qk_group("k", 0, 1)
    mand_post[0].extend(
        Job(1024, (lambda p, k: (lambda: emit_v_group(p, k)))(0, kt))
        for kt in (0, 1)
    )

    # ---------------- main pipeline ----------------------------------------
    BUDGET = 2200
    for r in range(NR):
        for job in mand_pre[r]:
            job.run()
        emit_scores(r)
        if r > 0:
            emit_pv(r - 1)
        if r % 8 == 0 and r > 0:
            s = r // 8 - 1
            emit_normalize(s)
            pair, qc = divmod(s, 4)
            if pair == 1:
                pending.extend((qc, qi, ec) for qi in range(4) for ec in range(2))
        for job in mand_post[r]:
            job.run()
        budget = BUDGET
        while budget > 0:
            while fq and fq[0][1].done:
                fq.popleft()
            if fq and fq[0][0] <= r:
                _, job = fq.popleft()
                budget -= job.cols
                job.run()
            elif pending:
                emit_outproj_unit()
                budget -= 1024
            else:
                break

    # ---------------- drain -------------------------------------------------
    emit_pv(NR - 1)
    zfin = {}
    for i in (0, 1):
        zr = sb.tile([65, QC], F32, tag="zr", bufs=2, name=f"zrf{i}")
        nc.vector.tensor_copy(zr[64:65, :], U[7][i][64:65, :])
        z = sb.tile([1, QC], F32, tag="z", bufs=2, name=f"zf{i}")
        nc.sync.dma_start(z[:], zr[64:65, :])
        zfin[i] = z
    for _, job in fq:
        job.run()
    tags = ["pj", "u", "u", "u"]
    i = 0
    while pending:
        emit_outproj_unit(tag=tags[i % 4])
        i += 1
    while half_pending:
        emit_outproj_half(tag=tags[i % 4])
        i += 1
    unf = sb.tile([P, QC], BF16, tag="un", bufs=8, name="UN3_1")
    for i in (0, 1):
        rz = sb.tile([1, QC], F32, tag="rz", bufs=2, name=f"rzf{i}")
        nc.vector.reciprocal(rz[:], zfin[i][:])
        rb = sb.tile([64, QC], F32, tag="rb", bufs=2, name=f"rbf{i}")
        nc.gpsimd.partition_broadcast(rb[:], rz[:], channels=64)
        if i == 0:
            nc.vector.tensor_mul(unf[0:64, :], U[7][i][0:64, :], rb[:])
        else:
            tmp = sb.tile([64, QC], BF16, tag="untmp", bufs=2, name="utf")
            nc.vector.tensor_mul(tmp[:], U[7][i][0:64, :], rb[:])
            nc.sync.dma_start(unf[64:128, :], tmp[:])
    del U[7]
    UN[(3, 1)] = unf

    # qc3 completions: all pair1 matmuls first (4 PSUM slots in flight),
    # adds/DMAs drain on DVE/sync concurrently.
    comp = []
    for n, (qi, ec) in enumerate([(a, b) for a in range(4) for b in range(2)]):
        tag = tags[n % 4]
        yp = ps.tile(
            [P, QC], F32, tag=tag, bufs=(1 if tag == "pj" else 3), name=f"yc{qi}_{ec}"
        )
        nc.tensor.matmul(
            yp[:],
            UN[(3, 1)][:, qi * P : (qi + 1) * P],
            woT[1][:, ec * QC : (ec + 1) * QC],
            start=True,
            stop=True,
        )
        comp.append((qi, ec, yp))
    for qi, ec, yp in comp:
        qt = 12 + qi
        ysb = YSB[qt]
        nc.vector.tensor_add(
            ysb[:, ec * QC : (ec + 1) * QC], yp[:], ysb[:, ec * QC : (ec + 1) * QC]
        )
        nc.sync.dma_start(
            io["y"][qt * P : (qt + 1) * P, ec * QC : (ec + 1) * QC],
            ysb[:, ec * QC : (ec + 1) * QC],
        )


def build_program():
    nc = bacc.Bacc(
        "TRN2", target_bir_lowering=False, debug=False, num_devices=NCORES
    )
    io = {
        "xq": nc.dram_tensor("xq", [NQC, P, CD, QC], BF16, kind="ExternalInput").ap(),
        "xk": nc.dram_tensor("xk", [NQC, P, CD, QC], BF16, kind="ExternalInput").ap(),
        "xv": nc.dram_tensor("xv", [NQC, P, CD, QC], BF16, kind="ExternalInput").ap(),
        "wq": nc.dram_tensor("wq", [2, P, CD, P], BF16, kind="ExternalInput").ap(),
        "wk": nc.dram_tensor("wk", [2, P, CD, P], BF16, kind="ExternalInput").ap(),
        "wv": nc.dram_tensor("wv", [2, P, CD, P], BF16, kind="ExternalInput").ap(),
        "wo": nc.dram_tensor("wo", [2, P, D], BF16, kind="ExternalInput").ap(),
        "bq": nc.dram_tensor("bq", [P, 2], F32, kind="ExternalInput").ap(),
        "ones2": nc.dram_tensor("ones2", [P, 2], BF16, kind="ExternalInput").ap(),
        "y": nc.dram_tensor("y", [S, D], BF16, kind="ExternalOutput").ap(),
    }
    with tile.TileContext(nc) as tc:
        with ExitStack() as ctx:
            _body(ctx, tc, io)
    nc.compile()
    return nc


_CACHE = {}


def _get_program():
    if "nc" not in _CACHE:
        _CACHE["nc"] = build_program()
    return _CACHE["nc"]


def make_in_maps(inputs):
    q = np.asarray(inputs["query"], np.float32)
    k = np.asarray(inputs["key"], np.float32)
    v = np.asarray(inputs["value"], np.float32)
    W_q = np.asarray(inputs["W_q"], np.float32)
    W_k = np.asarray(inputs["W_k"], np.float32)
    W_v = np.asarray(inputs["W_v"], np.float32)
    W_o = np.asarray(inputs["W_o"], np.float32)
    b_q = np.asarray(inputs["b_q"], np.float32)

    bf = ml_dtypes.bfloat16

    def xblocks(x):  # [S, D] activations -> [blk, p, c, s] with x.T blocked
        xt = np.ascontiguousarray(x.T).astype(bf)  # [D, S]
        return np.ascontiguousarray(
            xt.reshape(CD, P, NQC, QC).transpose(2, 1, 0, 3)
        )

    def wblocks(w_sl):  # [D, 256] (= W[sl].T) -> [pr, p, c, d]
        return np.ascontiguousarray(
            w_sl.reshape(CD, P, 2, P).transpose(2, 1, 0, 3).astype(bf)
        )

    xq = [xblocks(q[b]) for b in range(B)]
    xk = [xblocks(k[b]) for b in range(B)]
    xv = [xblocks(v[b]) for b in range(B)]

    in_maps = []
    for core in range(NCORES):
        b, g = divmod(core, NG)
        sl = slice(g * DG, (g + 1) * DG)
        in_maps.append(
            {
                "xq": xq[b],
                "xk": xk[b],
                "xv": xv[b],
                "wq": wblocks(W_q[sl, :].T),
                "wk": wblocks(W_k[sl, :].T),
                "wv": wblocks(W_v[sl, :].T),
                "wo": np.ascontiguousarray(
                    W_o[:, sl].T.reshape(2, P, D).astype(bf)
                ),
                "bq": np.ascontiguousarray(b_q[sl].reshape(2, P).T),
                "ones2": np.ones((P, 2), bf),
            }
        )
    return in_maps


def kernel(**inputs):
    from concourse.bass_utils import run_bass_kernel_spmd

    nc = _get_program()
    in_maps = make_in_maps(inputs)
    trace = bool(int(os.environ.get("MHA_TRACE", "0")))
    res = run_bass_kernel_spmd(nc, in_maps, list(range(NCORES)), trace=trace)
    _CACHE["last_results"] = res

    W_o = np.asarray(inputs["W_o"], np.float64)
    b_o = np.asarray(inputs["b_o"], np.float64)
    b_v = np.asarray(inputs["b_v"], np.float64)
    out = np.zeros((B, S, D), np.float32)
    for core in range(NCORES):
        b = core // NG
        out[b] += res.results[core]["y"].astype(np.float32)
    # b_v and b_o commute with the attention average / output projection.
    out += (b_o + b_v @ W_o.T).astype(np.float32)[None, None, :]
    return out


# revision 21
# speedup vs baseline: 1.1238x; 1.0047x over previous
"""Multi-head attention (B=2, S=2048, D=1024, H=16) on 8 Trainium2 cores.

Sharding: core = 4*b + g  (b = batch 0..1, g = head-group 0..3, 4 heads each).
Heads are processed in pairs; pair p covers the group's d-dims [128p, 128p+128).

Schedule: the scalar engine's exp stream (128 activations of [128,1024],
~172us) is the critical resource.  A short DMA-led preamble projects only
K(pair0), Q(pair0,qc0), V(kt0,1); then 64 pipelined rounds (one per
(pair, qc, k-group)) keep ACT continuously busy:

  round r: [forced proj groups] scores(r) -> exp(r) -> PV(r-1)
           [normalize at sweep boundaries] [filler: proj / out-proj]

All other projections (K pair1, remaining Q, V) and the output projection
run as PE filler inside the rounds' slack so the tensor engine never idles
(and stays at full DVFS pstate).  Sweep order is pair-major so pair1's
weights/projections have 4 sweeps of slack to materialize.

Exactness notes: b_k only shifts each softmax row uniformly -> dropped.
b_v and b_o commute with softmax-average -> folded into the host reduce.
b_q is applied on-device (fused into the Q PSUM->SBUF copy).
"""

import os
from collections import defaultdict, deque
from contextlib import ExitStack

import ml_dtypes
import numpy as np

import concourse.bass as bass
import concourse.tile as tile
from concourse import bacc, mybir

B, S, D = 2, 2048, 1024
H, DH = 16, 64
NCORES = 8
NG = 4                  # head-group shards
DG = D // NG            # 256 dims per head-group (4 heads, 2 pairs)
P = 128
QC = 512                # q-chunk width
NQC = S // QC           # 4
NKT = S // P            # 16 k-tiles of 128
CD = D // P             # 8 contraction tiles for the projections
NR = 64                 # pipeline rounds: 2 pairs x 4 qc x 8 k-groups
F32 = mybir.dt.float32
BF16 = mybir.dt.bfloat16
AF = mybir.ActivationFunctionType
SCALE = 1.0 / float(np.sqrt(D))


def _body(ctx: ExitStack, tc: "tile.TileContext", io: dict):
    nc = tc.nc
    ctx.enter_context(nc.allow_low_precision(reason="bf16 matmul pipeline"))
    sb = ctx.enter_context(tc.tile_pool(name="sb", bufs=1))
    ps = ctx.enter_context(tc.tile_pool(name="ps", bufs=1, space="PSUM"))

    # ---------------- DMA: inputs stream in consumption order --------------
    xk_sb, xq_sb, xv_sb = {}, {}, {}
    w_sb = {}

    def dma_x(dst_map, key, idx):
        t = sb.tile([P, CD, QC], BF16, tag="x", bufs=12, name=f"{key}{idx}")
        # two halves so projection groups can start on the first half
        nc.sync.dma_start(t[:, 0:4, :], io[key][idx, :, 0:4, :])
        nc.sync.dma_start(t[:, 4:8, :], io[key][idx, :, 4:8, :])
        dst_map[idx] = t

    def dma_w(kind, pr):
        t = sb.tile([P, CD, P], BF16, tag="w", bufs=6, name=f"w{kind}{pr}")
        nc.sync.dma_start(t[:], io[f"w{kind}"][pr])
        w_sb[(kind, pr)] = t

    dma_w("k", 0)
    dma_x(xk_sb, "xk", 0)
    dma_w("q", 0)
    bq = sb.tile([P, 2], F32, tag="bq", bufs=1, name="bq")
    nc.sync.dma_start(bq[:], io["bq"])
    ones2 = sb.tile([P, 2], BF16, tag="ones2", bufs=1, name="ones2")
    nc.sync.dma_start(ones2[:], io["ones2"])
    dma_x(xk_sb, "xk", 1)
    dma_x(xq_sb, "xq", 0)
    dma_w("v", 0)
    dma_x(xv_sb, "xv", 0)
    dma_x(xv_sb, "xv", 1)
    dma_x(xq_sb, "xq", 1)
    dma_x(xk_sb, "xk", 2)
    dma_x(xv_sb, "xv", 2)
    dma_x(xk_sb, "xk", 3)
    dma_x(xv_sb, "xv", 3)
    dma_w("k", 1)
    dma_w("q", 1)
    dma_x(xq_sb, "xq", 2)
    dma_w("v", 1)
    dma_x(xq_sb, "xq", 3)
    woT = []
    for pr in range(2):
        t = sb.tile([P, D], BF16, tag="wo", bufs=2, name=f"woT{pr}")
        nc.sync.dma_start(t[:], io["wo"][pr])
        woT.append(t)

    # ---------------- projection / out-proj emitters -----------------------
    KT, QT, V, UN, YSB = {}, {}, {}, {}, {}

    def emit_qk_group(kind, pr, idx):
        w = w_sb[(kind, pr)]
        x = (xk_sb if kind == "k" else xq_sb)[idx]
        pg = ps.tile([P, QC], F32, tag="pj", bufs=1, name=f"pg{kind}{pr}{idx}")
        for c in range(CD):
            nc.tensor.matmul(
                pg[:], w[:, c, :], x[:, c, :], start=(c == 0), stop=(c == CD - 1)
            )
        t = sb.tile([P, QC], BF16, tag=f"{kind}t", bufs=8, name=f"{kind}T{pr}_{idx}")
        if kind == "q":
            nc.vector.tensor_scalar_add(t[:], pg[:], bq[:, pr : pr + 1])
            QT[(pr, idx)] = t
        else:
            nc.vector.tensor_copy(t[:], pg[:])
            KT[(pr, idx)] = t

    def emit_v_group(pair, kt):
        sc, off = divmod(kt, 4)
        x = xv_sb[sc]
        pg = ps.tile([P, P], F32, tag="pj", bufs=1, name=f"pgv{pair}{kt}")
        for c in range(CD):
            nc.tensor.matmul(
                pg[:],
                x[:, c, off * P : (off + 1) * P],
                w_sb[("v", pair)][:, c, :],
                start=(c == 0),
                stop=(c == CD - 1),
            )
        vt = sb.tile([P, 2, DH + 1], BF16, tag="v", bufs=32, name=f"V{pair}_{kt}")
        nc.vector.tensor_copy(vt[:, :, 0:DH], pg[:].rearrange("p (i d) -> p i d", i=2))
        nc.vector.tensor_copy(vt[:, :, DH : DH + 1], ones2[:, :, None])
        V[(pair, kt)] = vt

    pending = deque()
    half_pending = deque()

    def emit_outproj_half(tag="pj"):
        # pair0 half of a qc3 unit; bf16 partial parked in SBUF
        qi, ec = half_pending.popleft()
        qt = 12 + qi
        if ec == 0:
            YSB[qt] = sb.tile([P, D], BF16, tag="y3", bufs=4, name=f"YP{qt}")
        yp = ps.tile(
            [P, QC], F32, tag=tag, bufs=(1 if tag == "pj" else 3), name=f"yh{qt}_{ec}"
        )
        nc.tensor.matmul(
            yp[:],
            UN[(3, 0)][:, qi * P : (qi + 1) * P],
            woT[0][:, ec * QC : (ec + 1) * QC],
            start=True,
            stop=True,
        )
        nc.vector.tensor_copy(YSB[qt][:, ec * QC : (ec + 1) * QC], yp[:])

    def emit_outproj_unit(tag="pj"):
        qc, qi, ec = pending.popleft()
        qt = qc * 4 + qi
        if ec == 0:
            YSB[qt] = sb.tile([P, D], BF16, tag="y", bufs=4, name=f"Y{qt}")
        ysb = YSB[qt]
        yp = ps.tile([P, QC], F32, tag=tag, bufs=(1 if tag == "pj" else 3), name=f"yp{qt}_{ec}")
        for pr in range(2):
            nc.tensor.matmul(
                yp[:],
                UN[(qc, pr)][:, qi * P : (qi + 1) * P],
                woT[pr][:, ec * QC : (ec + 1) * QC],
                start=(pr == 0),
                stop=(pr == 1),
            )
        nc.vector.tensor_copy(ysb[:, ec * QC : (ec + 1) * QC], yp[:])
        if ec == 1:
            nc.sync.dma_start(io["y"][qt * P : (qt + 1) * P, :], ysb[:])

    # ---------------- attention round emitters -----------------------------
    PTs, U = {}, {}

    def sweep_of(r):
        s = r // 8
        pair, qc = divmod(s, 4)
        return s, pair, qc, r % 8

    def emit_scores(r):
        _, pair, qc, kg = sweep_of(r)
        for i in (0, 1):
            lo = 64 * i
            st = ps.tile([P, 2, QC], F32, tag="st", bufs=2, name=f"st{r}_{i}")
            for kk in (0, 1):
                kt = kg * 2 + kk
                sc, off = divmod(kt, 4)
                nc.tensor.matmul(
                    st[:, kk, :],
                    KT[(pair, sc)][lo : lo + 64, off * P : (off + 1) * P],
                    QT[(pair, qc)][lo : lo + 64, :],
                    start=True,
                    stop=True,
                    tile_position=(lo, 0),
                )
            pt = sb.tile([P, 2, QC], BF16, tag="pt", bufs=4, name=f"pt{r}_{i}")
            nc.scalar.activation(
                pt[:].rearrange("p a b -> p (a b)"),
                st[:].rearrange("p a b -> p (a b)"),
                AF.Exp,
                scale=SCALE,
            )
            PTs[(r, i)] = pt

    def emit_pv(r):
        s, pair, qc, kg = sweep_of(r)
        if kg == 0:
            U[s] = [
                ps.tile([P, QC], F32, tag="u", bufs=3, name=f"U{s}_{i}") for i in (0, 1)
            ]
        for i in (0, 1):
            pt = PTs.pop((r, i))
            for kk in (0, 1):
                kt = kg * 2 + kk
                nc.tensor.matmul(
                    U[s][i][0:65, :],
                    V[(pair, kt)][:, i, :],
                    pt[:, kk, :],
                    start=(kg == 0 and kk == 0),
                    stop=(kg == 7 and kk == 1),
                )

    def emit_normalize(s):
        pair, qc = divmod(s, 4)
        un = sb.tile([P, QC], BF16, tag="un", bufs=8, name=f"UN{qc}_{pair}")
        for i in (0, 1):
            u = U[s][i]
            zr = sb.tile([65, QC], F32, tag="zr", bufs=2, name=f"zr{s}_{i}")
            nc.vector.tensor_copy(zr[64:65, :], u[64:65, :])
            z = sb.tile([1, QC], F32, tag="z", bufs=2, name=f"z{s}_{i}")
            nc.sync.dma_start(z[:], zr[64:65, :])
            rz = sb.tile([1, QC], F32, tag="rz", bufs=2, name=f"rz{s}_{i}")
            nc.vector.reciprocal(rz[:], z[:])
            rb = sb.tile([64, QC], F32, tag="rb", bufs=2, name=f"rb{s}_{i}")
            nc.gpsimd.partition_broadcast(rb[:], rz[:], channels=64)
            if i == 0:
                nc.vector.tensor_mul(un[0:64, :], u[0:64, :], rb[:])
            else:
                tmp = sb.tile([64, QC], BF16, tag="untmp", bufs=2, name=f"ut{s}")
                nc.vector.tensor_mul(tmp[:], u[0:64, :], rb[:])
                nc.sync.dma_start(un[64:128, :], tmp[:])
        del U[s]
        UN[(qc, pair)] = un
        if pair == 0 and qc == 3:
            half_pending.extend((qi, ec) for qi in range(4) for ec in range(2))

    # ---------------- static schedule --------------------------------------
    # Fillers: (cols, closure); forced at their deadline round if not yet run.
    class Job:
        __slots__ = ("cols", "fn", "done")

        def __init__(self, cols, fn):
            self.cols, self.fn, self.done = cols, fn, False

        def run(self):
            if not self.done:
                self.done = True
                self.fn()

    def qk_job(kind, pr, idx):
        return Job(4096, lambda: emit_qk_group(kind, pr, idx))

    jobs = {}
    for kind, pr, idx in [
        ("q", 0, 1), ("q", 0, 2), ("q", 0, 3),
        ("k", 1, 0), ("k", 1, 1), ("k", 1, 2), ("k", 1, 3),
        ("q", 1, 0), ("q", 1, 1), ("q", 1, 2), ("q", 1, 3),
    ]:
        jobs[(kind, pr, idx)] = qk_job(kind, pr, idx)

    # EDF-ordered general filler queue with earliest-emission gates
    # (rounds before which the needed DMA has not landed yet).
    fq = deque(
        [
            (8, jobs[("q", 0, 1)]),
            (9, jobs[("k", 1, 0)]),
            (10, jobs[("q", 1, 0)]),
            (10, jobs[("q", 0, 2)]),
            (10, jobs[("k", 1, 1)]),
            (11, jobs[("k", 1, 2)]),
            (11, jobs[("k", 1, 3)]),
            (12, jobs[("q", 0, 3)]),
            (13, jobs[("q", 1, 1)]),
            (13, jobs[("q", 1, 2)]),
            (14, jobs[("q", 1, 3)]),
        ]
        + [(33 + j, Job(512, lambda: emit_outproj_half())) for j in range(8)]
    )

    # mand_pre: tiles this round's scores read -> must be emitted first.
    jobs[("k", 0, 2)] = qk_job("k", 0, 2)
    jobs[("k", 0, 3)] = qk_job("k", 0, 3)
    mand_pre = defaultdict(list)
    mand_pre[2].append(jobs[("k", 0, 2)])
    mand_pre[4].append(jobs[("k", 0, 3)])
    mand_pre[8].append(jobs[("q", 0, 1)])
    mand_pre[16].append(jobs[("q", 0, 2)])
    mand_pre[24].append(jobs[("q", 0, 3)])
    mand_pre[32].append(jobs[("k", 1, 0)])
    mand_pre[32].append(jobs[("q", 1, 0)])
    mand_pre[34].append(jobs[("k", 1, 1)])
    mand_pre[36].append(jobs[("k", 1, 2)])
    mand_pre[38].append(jobs[("k", 1, 3)])
    mand_pre[40].append(jobs[("q", 1, 1)])
    mand_pre[48].append(jobs[("q", 1, 2)])
    mand_pre[56].append(jobs[("q", 1, 3)])

    # mand_post: V just-in-time (consumed by PV one/two rounds later).
    mand_post = defaultdict(list)
    for r in range(7):  # pair0 kt2..15
        mand_post[r].extend(
            Job(1024, (lambda p, k: (lambda: emit_v_group(p, k)))(0, kt))
            for kt in (2 * r + 2, 2 * r + 3)
        )
    for j, r in enumerate(range(24, 32)):  # pair1 kt0..15
        mand_post[r].extend(
            Job(1024, (lambda p, k: (lambda: emit_v_group(p, k)))(1, kt))
            for kt in (2 * j, 2 * j + 1)
        )

    # ---------------- preamble ---------------------------------------------
    emit_qk_group("k", 0, 0)
    emit_qk_group("k", 0, 1)
    emit_qk_group("q", 0, 0)
    mand_post[0].extend(
        Job(1024, (lambda p, k: (lambda: emit_v_group(p, k)))(0, kt))
        for kt in (0, 1)
    )

    # ---------------- main pipeline ----------------------------------------
    BUDGET = 2200
    for r in range(NR):
        for job in mand_pre[r]:
            job.run()
        emit_scores(r)
        if r > 0:
            emit_pv(r - 1)
        if r % 8 == 0 and r > 0:
            s = r // 8 - 1
            emit_normalize(s)
            pair, qc = divmod(s, 4)
            if pair == 1:
                pending.extend((qc, qi, ec) for qi in range(4) for ec in range(2))
        for job in mand_post[r]:
            job.run()
        budget = BUDGET
        while budget > 0:
            while fq and fq[0][1].done:
                fq.popleft()
            if fq and fq[0][0] <= r:
                _, job = fq.popleft()
                budget -= job.cols
                job.run()
            elif pending:
                emit_outproj_unit()
                budget -= 1024
            else:
                break

    # ---------------- drain -------------------------------------------------
    emit_pv(NR - 1)
    emit_normalize(7)
    for _, job in fq:
        job.run()
    tags = ["pj", "u", "u", "u"]
    i = 0
    while pending:
        emit_outproj_unit(tag=tags[i % 4])
        i += 1
    while half_pending:
        emit_outproj_half(tag=tags[i % 4])
        i += 1
    # qc3 completions: all pair1 matmuls first (4 PSUM slots in flight),
    # adds/DMAs drain on DVE/sync concurrently.
    comp = []
    for n, (qi, ec) in enumerate([(a, b) for a in range(4) for b in range(2)]):
        tag = tags[n % 4]
        yp = ps.tile(
            [P, QC], F32, tag=tag, bufs=(1 if tag == "pj" else 3), name=f"yc{qi}_{ec}"
        )
        nc.tensor.matmul(
            yp[:],
            UN[(3, 1)][:, qi * P : (qi + 1) * P],
            woT[1][:, ec * QC : (ec + 1) * QC],
            start=True,
            stop=True,
        )
        comp.append((qi, ec, yp))
    for qi, ec, yp in comp:
        qt = 12 + qi
        ysb = YSB[qt]
        nc.vector.tensor_add(
            ysb[:, ec * QC : (ec + 1) * QC], yp[:], ysb[:, ec * QC : (ec + 1) * QC]
        )
        nc.sync.dma_start(
            io["y"][qt * P : (qt + 1) * P, ec * QC : (ec + 1) * QC],
            ysb[:, ec * QC : (ec + 1) * QC],
        )


def build_program():
    nc = bacc.Bacc(
        "TRN2", target_bir_lowering=False, debug=False, num_devices=NCORES
    )
    io = {
        "xq": nc.dram_tensor("xq", [NQC, P, CD, QC], BF16, kind="ExternalInput").ap(),
        "xk": nc.dram_tensor("xk", [NQC, P, CD, QC], BF16, kind="ExternalInput").ap(),
        "xv": nc.dram_tensor("xv", [NQC, P, CD, QC], BF16, kind="ExternalInput").ap(),
        "wq": nc.dram_tensor("wq", [2, P, CD, P], BF16, kind="ExternalInput").ap(),
        "wk": nc.dram_tensor("wk", [2, P, CD, P], BF16, kind="ExternalInput").ap(),
        "wv": nc.dram_tensor("wv", [2, P, CD, P], BF16, kind="ExternalInput").ap(),
        "wo": nc.dram_tensor("wo", [2, P, D], BF16, kind="ExternalInput").ap(),
        "bq": nc.dram_tensor("bq", [P, 2], F32, kind="ExternalInput").ap(),
        "ones2": nc.dram_tensor("ones2", [P, 2], BF16, kind="ExternalInput").ap(),
        "y": nc.dram_tensor("y", [S, D], BF16, kind="ExternalOutput").ap(),
    }
    with tile.TileContext(nc) as tc:
        with ExitStack() as ctx:
            _body(ctx, tc, io)
    nc.compile()
    return nc


_CACHE = {}


def _get_program():
    if "nc" not in _CACHE:
        _CACHE["nc"] = build_program()
    return _CACHE["nc"]


def make_in_maps(inputs):
    q = np.asarray(inputs["query"], np.float32)
    k = np.asarray(inputs["key"], np.float32)
    v = np.asarray(inputs["value"], np.float32)
    W_q = np.asarray(inputs["W_q"], np.float32)
    W_k = np.asarray(inputs["W_k"], np.float32)
    W_v = np.asarray(inputs["W_v"], np.float32)
    W_o = np.asarray(inputs["W_o"], np.float32)
    b_q = np.asarray(inputs["b_q"], np.float32)

    bf = ml_dtypes.bfloat16

    def xblocks(x):  # [S, D] activations -> [blk, p, c, s] with x.T blocked
        xt = np.ascontiguousarray(x.T).astype(bf)  # [D, S]
        return np.ascontiguousarray(
            xt.reshape(CD, P, NQC, QC).transpose(2, 1, 0, 3)
        )

    def wblocks(w_sl):  # [D, 256] (= W[sl].T) -> [pr, p, c, d]
        return np.ascontiguousarray(
            w_sl.reshape(CD, P, 2, P).transpose(2, 1, 0, 3).astype(bf)
        )

    xq = [xblocks(q[b]) for b in range(B)]
    xk = [xblocks(k[b]) for b in range(B)]
    xv = [xblocks(v[b]) for b in range(B)]

    in_maps = []
    for core in range(NCORES):
        b, g = divmod(core, NG)
        sl = slice(g * DG, (g + 1) * DG)
        in_maps.append(
            {
                "xq": xq[b],
                "xk": xk[b],
                "xv": xv[b],
                "wq": wblocks(W_q[sl, :].T),
                "wk": wblocks(W_k[sl, :].T),
                "wv": wblocks(W_v[sl, :].T),
                "wo": np.ascontiguousarray(
                    W_o[:, sl].T.reshape(2, P, D).astype(bf)
                ),
                "bq": np.ascontiguousarray(b_q[sl].reshape(2, P).T),
                "ones2": np.ones((P, 2), bf),
            }
        )
    return in_maps


def kernel(**inputs):
    from concourse.bass_utils import run_bass_kernel_spmd

    nc = _get_program()
    in_maps = make_in_maps(inputs)
    trace = bool(int(os.environ.get("MHA_TRACE", "0")))
    res = run_bass_kernel_spmd(nc, in_maps, list(range(NCORES)), trace=trace)
    _CACHE["last_results"] = res

    W_o = np.asarray(inputs["W_o"], np.float64)
    b_o = np.asarray(inputs["b_o"], np.float64)
    b_v = np.asarray(inputs["b_v"], np.float64)
    out = np.zeros((B, S, D), np.float32)
    for core in range(NCORES):
        b = core // NG
        out[b] += res.results[core]["y"].astype(np.float32)
    # b_v and b_o commute with the attention average / output projection.
    out += (b_o + b_v @ W_o.T).astype(np.float32)[None, None, :]
    return out
